# revision 49
# baseline (speedup 1.0000x reference)
"""AdaAugment Trainium2 kernel: reflect-pad + FIR up2 + affine bilinear warp + FIR down2.

Self-contained. Strategy (per NeuronCore, data-parallel over batch: 4 images/core):
 - host: reflect pad, banded FIR matrices, per-tile warp plans (indices/weights consts)
 - device: x-FIRs on DVE (strided taps), y-FIRs on PE (banded matmuls),
   warp via gpsimd indirect_copy gathers over DMA'd U windows, bilinear lerp on DVE,
   weights from iota + per-partition affine (bit-exact host mirror).
One SPMD graph for all 8 cores; all per-image geometry is input data.
"""
import sys, os
for p in ("/opt/trn_rl_repo", "/root/.axon_site/_ro/trn_rl_repo"):
    if os.path.isdir(p) and p not in sys.path:
        sys.path.insert(0, p)
import numpy as np

F32 = np.float32
H = W = 256
FW = 12
HZ_PAD = 3
MARGIN = 38
P = H + 2 * MARGIN            # 332
UH = UW = 664
WT = (H + 2 * HZ_PAD) * 2     # 524
TY, TX = 32, 66
GY, GX = 17, 8
WTY, WTX = GY * TY, GX * TX   # 544, 528
NIDX = TY * TX                # 2112
SW = NIDX // 16               # 132 wrapped idx cols
NB = 4 * GY                   # 68 batches per core
ZOFF = F32(1024.0)
NIMG = 4                      # images per core


# ---------------- host planning (mirrors device fp32 ops bit-exactly) --------

def affine_params(theta, log_s, tx, ty):
    N = theta.shape[0]
    s = np.exp(log_s).astype(F32)
    c, sn = np.cos(theta).astype(F32), np.sin(theta).astype(F32)
    A = np.zeros((N, 2, 3), F32)
    for i in range(N):
        rot = np.array([[c[i], sn[i], 0], [-sn[i], c[i], 0], [0, 0, 1]], F32)
        scl = np.array([[1 / s[i], 0, 0], [0, 1 / s[i], 0], [0, 0, 1]], F32)
        trn = np.array([[1, 0, -tx[i] * W], [0, 1, -ty[i] * H], [0, 0, 1]], F32)
        g = (scl @ rot @ trn).astype(F32)
        g = (np.array([[2, 0, 0], [0, 2, 0], [0, 0, 1]], F32) @ g
             @ np.array([[.5, 0, 0], [0, .5, 0], [0, 0, 1]], F32)).astype(F32)
        g = (np.array([[1, 0, -.5], [0, 1, -.5], [0, 0, 1]], F32) @ g
             @ np.array([[1, 0, .5], [0, 1, .5], [0, 0, 1]], F32)).astype(F32)
        g = (np.array([[2.0 / UW, 0, 0], [0, 2.0 / UH, 0], [0, 0, 1]], F32) @ g
             @ np.array([[WT / 2.0, 0, 0], [0, WT / 2.0, 0], [0, 0, 1]], F32)).astype(F32)
        A[i] = g[:2, :]
    return A


def pixel_affine(A):
    Ad = A.astype(np.float64)
    ax = Ad[0, 0] * UW / WT
    bx = Ad[0, 1] * UW / WT
    cx = (Ad[0, 0] * (1.0 / WT - 1.0) + Ad[0, 1] * (1.0 / WT - 1.0) + Ad[0, 2] + 1.0) * UW / 2.0 - 0.5
    ay = Ad[1, 0] * UW / WT
    by = Ad[1, 1] * UW / WT
    cy = (Ad[1, 0] * (1.0 / WT - 1.0) + Ad[1, 1] * (1.0 / WT - 1.0) + Ad[1, 2] + 1.0) * UH / 2.0 - 0.5
    return F32(ax), F32(bx), F32(cx), F32(ay), F32(by), F32(cy)


def fir_up_matrix(f):
    f2 = np.asarray(f, np.float64) * 2.0
    p0 = (FW + 1) // 2
    B = np.zeros((P, 2 * P), np.float64)
    for m in range(P):
        jlo, jhi = p0 + 2 * m - (FW - 1), p0 + 2 * m
        for j in range(max(jlo, 0), min(jhi + 1, 2 * P)):
            B[m, j] = f2[FW - 1 - (p0 + 2 * m - j)]
    return B.astype(F32)


def fir_down_matrix(f):
    fd = np.asarray(f, np.float64)
    B = np.zeros((WT, H), np.float64)
    for j in range(H):
        for t in range(FW):
            m = 2 * j + 1 + t
            if 0 <= m < WT:
                B[m, j] = fd[t]
    return B.astype(F32)


def reflect_pad(img):
    return np.pad(img, ((0, 0), (0, 0), (MARGIN, MARGIN), (MARGIN, MARGIN)), mode="reflect")


def plan_image(A):
    ax, bx, cx, ay, by, cy = pixel_affine(A)
    tiles = []
    ly = np.arange(TY, dtype=F32)[:, None]
    lx = np.arange(TX, dtype=F32)[None, :]
    for ty in range(GY):
        for tg in range(GX):
            yo0, xo0 = ty * TY, tg * TX
            Cx = F32(F32(F32(ax * xo0) + F32(bx * yo0)) + F32(cx + float(ZOFF)))
            Cy = F32(F32(F32(ay * xo0) + F32(by * yo0)) + F32(cy + float(ZOFF)))
            # device order: tA = f32(f32(ly*b) + C); z = f32(f32(lx*a) + tA)
            zx = np.float32(np.float32(lx * ax) + np.float32(np.float32(ly * bx) + Cx))
            zy = np.float32(np.float32(lx * ay) + np.float32(np.float32(ly * by) + Cy))
            wx = np.fmod(zx, F32(1.0))
            wy = np.fmod(zy, F32(1.0))
            ix0 = np.floor(zx).astype(np.int64) - int(ZOFF)
            iy0 = np.floor(zy).astype(np.int64) - int(ZOFF)
            tiles.append(dict(ty=ty, tg=tg, ix0=ix0, iy0=iy0, wx=wx, wy=wy,
                              consts=(ax, bx, Cx, ay, by, Cy)))
    return tiles


def window_extents(all_tiles):
    WRM = WCM = 8
    for tiles in all_tiles:
        for t in tiles:
            vx = (t["ix0"] >= -1) & (t["ix0"] <= UW - 1)
            vy = (t["iy0"] >= -1) & (t["iy0"] <= UH - 1)
            use = vx & vy
            if use.any():
                c0i = max(int(t["ix0"][use].min()), 0)
                c1i = min(int(t["ix0"][use].max()) + 1, UW - 1)
                r0i = max(int(t["iy0"][use].min()), 0)
                r1i = min(int(t["iy0"][use].max()) + 1, UH - 1)
                WRM = max(WRM, r1i - r0i + 1)
                WCM = max(WCM, c1i - c0i + 1)
                t["r0"], t["c0"] = r0i, c0i
            else:
                t["r0"], t["c0"] = 0, 0
    return WRM, WCM


def finalize_tiles(all_tiles, WRM, WCM):
    WRF, WCF = WRM + 4, WCM + 4
    for tiles in all_tiles:
        for t in tiles:
            r0 = min(t["r0"], UH - WRM)
            c0 = min(t["c0"], UW - WCM)
            t["r0"], t["c0"] = r0, c0
            ix0, iy0 = t["ix0"], t["iy0"]
            lc = ix0 - c0 + 2
            lr = iy0 - r0 + 2
            lc = np.where(ix0 < c0 - 1, 0, lc)
            lc = np.where(ix0 == c0 - 1, 1, lc)
            lc = np.where(ix0 > c0 + WCM - 1, WCF - 2, lc)
            lr = np.where(iy0 < r0 - 1, 0, lr)
            lr = np.where(iy0 == r0 - 1, 1, lr)
            lr = np.where(iy0 > r0 + WRM - 1, WRF - 2, lr)
            # quad index into the row-pair interleaved window (2 elems per slot)
            t["idxA"] = (2 * (lr * WCF + lc)).astype(np.uint16).ravel()
    return WRF, WCF


def wrap16(idx_flat):
    """Wrap the full index stream into [16, n/16] (single indirect_copy call)."""
    return idx_flat.reshape(idx_flat.shape[0] // 16, 16).T


def tap_structure(B, up):
    """Extract (offsets, coeffs) per output parity from a banded FIR matrix.
    up=2: out col j=2q+par taps rows q+dm; up=1(down): out col j taps rows 2j+dm."""
    taps = []
    if up == 2:
        for par in (0, 1):
            q0 = B.shape[0] // 2
            col = B[:, 2 * q0 + par]
            rows = np.nonzero(col)[0]
            taps.append([(int(r - q0), float(col[r])) for r in rows])
    else:
        j0 = B.shape[1] // 2
        col = B[:, j0]
        rows = np.nonzero(col)[0]
        taps.append([(int(r - 2 * j0), float(col[r])) for r in rows])
    return taps


# ---------------- device graph ----------------------------------------------

def build_graph(WRF, WCF, WRM, WCM, up_taps, dn_taps):
    import concourse.bass as bass
    import concourse.bacc as bacc
    import concourse.mybir as mybir
    from concourse.tile import TileContext

    dt = mybir.dt
    ALU = mybir.AluOpType
    ACTF = mybir.ActivationFunctionType
    FL = dt.float32
    BF = dt.bfloat16

    nc = bacc.Bacc("TRN2", target_bir_lowering=False, debug=False, num_devices=8)
    nc.disable_value_cache = True
    xpad_t = nc.dram_tensor("xpad", [NIMG, 3, P, P], FL, kind="ExternalInput")
    buy_t = nc.dram_tensor("buy", [3, 128, UH], FL, kind="ExternalInput")
    bdy_t = nc.dram_tensor("bdy", [5, 128, H], FL, kind="ExternalInput")
    idx_t = nc.dram_tensor("idx", [NB, 128, SW], dt.uint16, kind="ExternalInput")
    scal_t = nc.dram_tensor("scal", [NB, 6, 128], FL, kind="ExternalInput")
    offb_t = nc.dram_tensor("offb", [NB * 8], dt.int32, kind="ExternalInput")
    dbg = "ExternalOutput" if os.environ.get("ADA_DEBUG") == "1" else "Internal"
    u_dram = nc.dram_tensor("u_dbg", [NIMG * 3 * UH * UW + UH * UW], BF, kind=dbg)
    w2_dram = nc.dram_tensor("w2_dbg", [NIMG * 3 * WTY * WTX], BF, kind=dbg)
    out_t = nc.dram_tensor("out", [NIMG, 3, H, W], FL, kind="ExternalOutput")

    def dap(th, offset, dims):
        return bass.AP(th, int(offset), [list(d) for d in dims])

    with TileContext(nc) as tc:
        with tc.tile_pool(name="const", bufs=1) as cpool, \
             tc.tile_pool(name="psum", bufs=4, space="PSUM") as ppool:

            # ---- constants staged once ----
            scal_sb = cpool.tile([128, NB, 6], FL, tag="scal")
            nc.sync.dma_start(out=scal_sb[:, :, :], in_=dap(
                scal_t, 0, [(1, 128), (6 * 128, NB), (128, 6)]))
            iota_xf = cpool.tile([128, NIDX], BF, tag="iotaxf")
            iota_yf = cpool.tile([128, NIDX], BF, tag="iotayf")

            # single window staging buffer (guards zeroed once)
            wb0 = cpool.tile([128, WRF * WCF], BF, tag="wb0")
            nc.vector.memset(wb0[:, :], 0.0)
            wb_list = [wb0, wb0]
            # row-pair interleaved windows (double-buffered): wb2[2j]=wb[j],
            # wb2[2j+1]=wb[j+WCF] so one gather index fetches all 4 corners
            wb2a = cpool.tile([128, 2 * WRF * WCF], BF, tag="wb2a")
            nc.vector.memset(wb2a[:, :], 0.0)
            wb2b = cpool.tile([128, 2 * WRF * WCF], BF, tag="wb2b")
            nc.vector.memset(wb2b[:, :], 0.0)
            wb2_list = [wb2a, wb2b]

            # =================== phase 1: FIR up (per image) ===================
            fir_pool_ctx = tc.tile_pool(name="fir", bufs=1)
            fpool = fir_pool_ctx.__enter__()
            buy_sb = fpool.tile([128, 3, UH], FL, tag="buy")
            nc.sync.dma_start(out=buy_sb[:, :, :], in_=dap(
                buy_t, 0, [(UH, 128), (128 * UH, 3), (1, UH)]))
            iota_xi = fpool.tile([128, NIDX], dt.int32, tag="iotai")
            nc.gpsimd.iota(iota_xi[:, :], pattern=[[0, TY], [1, TX]], base=0,
                           channel_multiplier=0)
            nc.scalar.copy(out=iota_xf[:, :], in_=iota_xi[:, :])
            nc.gpsimd.iota(iota_xi[:, :], pattern=[[1, TY], [0, TX]], base=0,
                           channel_multiplier=0)
            nc.scalar.copy(out=iota_yf[:, :], in_=iota_xi[:, :])
            for img in range(NIMG):
                xpe = fpool.tile([128, 3, 3, P + 12], FL, tag="xpe")
                nc.vector.memset(xpe[:, :, :, :], 0.0)
                # load 332 rows into (blk, part): blk 0-1 full, blk 2 rows 0-75
                for blk in range(3):
                    pr = 128 if blk < 2 else P - 256
                    nc.sync.dma_start(
                        out=xpe[0:pr, blk, :, 6:6 + P],
                        in_=dap(xpad_t, img * 3 * P * P + blk * 128 * P,
                                [(P, pr), (P * P, 3), (1, P)]))
                # up-x on DVE: T1[.., par::2] = sum taps
                t1 = fpool.tile([128, 3, 3, UH], FL, tag="t1")
                for par in (0, 1):
                    for k, (dm, cf) in enumerate(up_taps[par]):
                        src = xpe[:, :, :, 6 + dm:6 + dm + P]
                        dst = t1[:, :, :, par::2]
                        if k == 0:
                            nc.vector.tensor_scalar(dst, src, float(cf), None, ALU.mult)
                        else:
                            nc.vector.scalar_tensor_tensor(
                                dst, src, float(cf), dst, ALU.mult, ALU.add)
                # up-y on PE: per M-tile, accumulate over K partition-blocks
                for mt in range(6):
                    ms, me = mt * 128, min(mt * 128 + 128, UH)
                    mm = me - ms
                    # K-window rows from Buy sparsity: out col j taps rows (j-par)/2+dm
                    r_lo = max(ms // 2 + min(d for d, _ in up_taps[0] + up_taps[1]), 0)
                    r_hi = min((me - 1) // 2 + max(d for d, _ in up_taps[0] + up_taps[1]), P - 1)
                    blks = list(range(r_lo // 128, r_hi // 128 + 1))
                    for ch in range(3):
                        for cs in (0, 512):
                            ce = min(cs + 512, UH)
                            nn = ce - cs
                            ps = ppool.tile([128, 512], FL, tag="ps_u")
                            for bi, b in enumerate(blks):
                                nc.tensor.matmul(
                                    ps[0:mm, 0:nn],
                                    buy_sb[:, b, ms:me],
                                    t1[:, b, ch, cs:ce],
                                    start=(bi == 0), stop=(bi == len(blks) - 1))
                            # evac + store (cast to bf16)
                            usb = fpool.tile([128, 512], BF, tag="usb")
                            nc.scalar.copy(out=usb[0:mm, 0:nn], in_=ps[0:mm, 0:nn])
                            nc.sync.dma_start(
                                out=dap(u_dram,
                                        (img * 3 + ch) * UH * UW + ms * UW + cs,
                                        [(UW, mm), (1, nn)]),
                                in_=usb[0:mm, 0:nn])

            fir_pool_ctx.__exit__(None, None, None)
            # =================== phase 2: warp (68 batches) ===================
            warp_pool_ctx = tc.tile_pool(name="warp", bufs=1)
            wpool = warp_pool_ctx.__enter__()
            prev_wdmas = []
            state = {}

            def head(b, tc=tc):
                # stage idx/offsets, fetch windows, build interleaved wb2.
                # Emitted one batch ahead so it overlaps batch b-1's gathers;
                # high_priority pulls it earlier in the scheduler's ordering.
                ctx = tc.high_priority(offset=80)
                ctx.__enter__()
                wb = wb_list[b % 2]
                idx_sb = wpool.tile([128, SW], dt.uint16, tag="idx", bufs=2)
                nc.scalar.dma_start(out=idx_sb[:, :], in_=dap(
                    idx_t, b * 128 * SW, [(SW, 128), (1, SW)]))
                offb_b = wpool.tile([128, 8], dt.int32, tag="offb_b", bufs=2)
                ob_dma = nc.scalar.dma_start(out=offb_b[0:1, :], in_=dap(
                    offb_t, b * 8, [(8, 1), (1, 8)]))
                if prev_wdmas:
                    bass._add_dep_helper(
                        ob_dma.ins, prev_wdmas[-1].ins, sync=True,
                        reason="offb slot reuse waits past prior register loads")
                u_ap = u_dram.ap()
                lds, vals = nc.values_load_multi_w_load_instructions(
                    offb_b[0:1, 0:8], engines=[mybir.EngineType.Activation],
                    min_val=0, max_val=(NIMG - 1) * 3 * UH * UW + UH * UW,
                    skip_runtime_bounds_check=True)
                if prev_wdmas:
                    for ld in lds:
                        bass._add_dep_helper(
                            ld.ins, prev_wdmas[-1].ins, sync=False,
                            reason="bound window-offset register liveness")
                wdmas = []
                for g in range(8):
                    src = u_ap[bass.ds(vals[g], 3 * UH * UW)].rearrange(
                        "(c r x) -> c r x", c=3, x=UW)[:, 0:WRM, 0:WCM]
                    wbv = wb[16 * g:16 * g + 3, :].rearrange(
                        "p (r c) -> p r c", c=WCF)[:, 2:2 + WRM, 2:2 + WCM]
                    wdmas.append(nc.scalar.dma_start(out=wbv, in_=src))
                # interleave: even elems <- wb[j] (ACT), odd <- wb[j+WCF] (DVE)
                wb2 = wb2_list[b % 2]
                wb2v = wb2[:, :].rearrange("p (a b) -> p a b", b=2)
                NW = WRF * WCF
                # even copy split across ACT/DVE halves so the build's wall
                # time (the inter-batch gap's long pole) is halved
                NH = (NW // 2) & ~1
                bld_e = nc.scalar.copy(
                    out=wb2v[:, 0:NH, 0:1].rearrange("p a b -> p (a b)"),
                    in_=wb[:, 0:NH])
                nc.vector.tensor_scalar(
                    wb2v[:, NH:NW, 0:1].rearrange("p a b -> p (a b)"),
                    wb[:, NH:], 1.0, None, ALU.mult)
                bld_o = nc.vector.tensor_scalar(
                    wb2v[:, 0:NW - WCF, 1:2].rearrange("p a b -> p (a b)"),
                    wb[:, WCF:], 1.0, None, ALU.mult)
                state[b] = (idx_sb, wb2)
                ctx.__exit__(None, None, None)
                return wdmas, bld_e, bld_o

            prev_wdmas, _, _ = head(0)
            for b in range(NB):
                img, ty = b // GY, b % GY
                # force next batch's wb2 build to schedule before this batch's
                # ACT/DVE compute so it hides under this batch's gathers
                if b + 1 < NB:
                    prev_wdmas, bld_e, bld_o = head(b + 1)
                    force_after = (bld_e, bld_o)
                else:
                    force_after = (None, None)
                idx_sb, wb2 = state.pop(b)
                # weights: zs = ax*iotaX + (bx*iotaY + Cx); w = frac(zs)
                tBb = wpool.tile([128, 2, NIDX], FL, tag="tB")
                tA = wpool.tile([128, NIDX], FL, tag="tA")
                for k, (o_a, o_b, o_c) in enumerate(((0, 1, 2), (3, 4, 5))):
                    # tA = b*iotaY + C        (ACT)
                    w_act = nc.scalar.activation(
                        tA[:, :], iota_yf[:, :], ACTF.Identity,
                        bias=scal_sb[:, b, o_c:o_c + 1],
                        scale=scal_sb[:, b, o_b:o_b + 1])
                    if False:
                        pass
                    # zs = (iotaX * a) + tA   (DVE stt)
                    nc.vector.scalar_tensor_tensor(
                        tBb[:, k, :], iota_xf[:, :], scal_sb[:, b, o_a:o_a + 1],
                        tA[:, :], ALU.mult, ALU.add)
                    # zf in tA: int-cast then float-cast in place
                    tAi = tA[:, :].bitcast(dt.int32)
                    nc.scalar.copy(out=tAi, in_=tBb[:, k, :])
                    nc.scalar.copy(out=tA[:, :], in_=tAi)
                    # fr = zs - zf (in place)
                    nc.vector.tensor_tensor(tBb[:, k, :], tBb[:, k, :], tA[:, :],
                                            ALU.subtract)
                # merged fixup for both weights: w = (fr<0) + fr  -> bf16
                wv = wpool.tile([128, 2, NIDX], BF, tag="wv")
                nc.vector.scalar_tensor_tensor(
                    wv[:, :, :], tBb[:, :, :], 0.0, tBb[:, :, :], ALU.is_lt, ALU.add)
                wx_t = wv[:, 0, :]
                wy_t = wv[:, 1, :]
                # quad gathers: one idx -> (v00,v10,v01,v11); dst cap 1024 elems
                gq = wpool.tile([128, NIDX, 4], BF, tag="gq", bufs=2)
                wb2d = wb2[:, :].rearrange("p (a b) -> p a b", b=4)
                for c0 in range(0, NIDX, 256):
                    c1 = min(c0 + 256, NIDX)
                    nc.gpsimd.indirect_copy(
                        gq[:, c0:c1, :], wb2d, idx_sb[:, c0 // 16:c1 // 16],
                        True)

                def ev(t, k):
                    return t[:, :, k:k + 1].rearrange("p a b -> p (a b)")
                # y-lerp both columns, then x-lerp
                tmp0 = tBb[:, 0, :]
                tmp1 = tBb[:, 1, :]
                l0 = nc.vector.tensor_tensor(tmp0, ev(gq, 1), ev(gq, 0),
                                             ALU.subtract)
                if force_after[1] is not None:
                    bass._add_dep_helper(
                        l0.ins, force_after[1].ins, sync=False,
                        reason="schedule next-batch wb2 build first on DVE")
                nc.vector.tensor_tensor(tmp0, tmp0, wy_t, ALU.mult)
                nc.vector.tensor_tensor(tmp0, tmp0, ev(gq, 0), ALU.add)
                nc.vector.tensor_tensor(tmp1, ev(gq, 3), ev(gq, 2), ALU.subtract)
                nc.vector.tensor_tensor(tmp1, tmp1, wy_t, ALU.mult)
                nc.vector.tensor_tensor(tmp1, tmp1, ev(gq, 2), ALU.add)
                nc.vector.tensor_tensor(tmp1, tmp1, tmp0, ALU.subtract)
                nc.vector.tensor_tensor(tmp1, tmp1, wx_t, ALU.mult)
                outt = wpool.tile([128, NIDX], BF, tag="outt", bufs=2)
                nc.vector.tensor_tensor(outt[:, :], tmp1, tmp0, ALU.add)
                # store stripe: one DMA per channel covering all 8 groups
                ov = outt[:, :].rearrange("(g s) (y x) -> g s y x", s=16, x=TX)
                for ch in range(3):
                    nc.sync.dma_start(
                        out=dap(w2_dram,
                                (img * 3 + ch) * WTY * WTX + ty * TY * WTX,
                                [(66, 8), (WTX, TY), (1, TX)]),
                        in_=ov[:, ch, :, :])

            warp_pool_ctx.__exit__(None, None, None)
            # =================== phase 3: FIR down (per image) =================
            dn_pool_ctx = tc.tile_pool(name="down", bufs=1)
            fpool = dn_pool_ctx.__enter__()
            bdy_sb = fpool.tile([128, 5, H], FL, tag="bdy")
            nc.sync.dma_start(out=bdy_sb[:, :, :], in_=dap(
                bdy_t, 0, [(H, 128), (128 * H, 5), (1, H)]))
            for img in range(NIMG):
                w2e = fpool.tile([128, 5, 3, WT], BF, tag="w2e")
                nc.vector.memset(w2e[:, :, :, :], 0.0)
                for blk in range(5):
                    pr = 128 if blk < 4 else WT - 512
                    nc.sync.dma_start(
                        out=w2e[0:pr, blk, :, :],
                        in_=dap(w2_dram, img * 3 * WTY * WTX + blk * 128 * WTX,
                                [(WTX, pr), (WTY * WTX, 3), (1, WT)]))
                # down-x on DVE (stride-2 taps)
                d1 = fpool.tile([128, 5, 3, H], FL, tag="d1")
                for k, (dm, cf) in enumerate(dn_taps[0]):
                    src = w2e[:, :, :, dm:dm + 2 * H:2]
                    if k == 0:
                        nc.vector.tensor_scalar(d1[:, :, :, :], src, float(cf), None, ALU.mult)
                    else:
                        nc.vector.scalar_tensor_tensor(
                            d1[:, :, :, :], src, float(cf), d1[:, :, :, :], ALU.mult, ALU.add)
                # down-y on PE
                dlo = min(d for d, _ in dn_taps[0])
                dhi = max(d for d, _ in dn_taps[0])
                for mt in range(2):
                    ms, me = mt * 128, mt * 128 + 128
                    r_lo = max(2 * ms + dlo, 0)
                    r_hi = min(2 * (me - 1) + dhi, WT - 1)
                    blks = list(range(r_lo // 128, r_hi // 128 + 1))
                    for ch in range(3):
                        ps = ppool.tile([128, 512], FL, tag="ps_o")
                        for bi, bb in enumerate(blks):
                            nc.tensor.matmul(
                                ps[0:128, 0:H],
                                bdy_sb[:, bb, ms:me],
                                d1[:, bb, ch, :],
                                start=(bi == 0), stop=(bi == len(blks) - 1))
                        ob = fpool.tile([128, H], FL, tag="ob")
                        nc.scalar.copy(out=ob[:, :], in_=ps[:, 0:H])
                        nc.sync.dma_start(
                            out=dap(out_t, (img * 3 + ch) * H * W + ms * W,
                                    [(W, 128), (1, H)]),
                            in_=ob[:, :])
            dn_pool_ctx.__exit__(None, None, None)

    nc.compile()
    return nc


# ---------------- entry point ------------------------------------------------

def kernel(**inputs):
    from concourse import bass_utils

    images = np.asarray(inputs["images"], np.float32)
    theta = np.asarray(inputs["theta"], np.float32)
    log_s = np.asarray(inputs["log_s"], np.float32)
    tx = np.asarray(inputs["tx"], np.float32)
    ty = np.asarray(inputs["ty"], np.float32)
    hz = np.asarray(inputs["hz_geom"], np.float32)
    N = images.shape[0]
    ncores = 8
    per = N // ncores

    A = affine_params(theta, log_s, tx, ty)
    xpad = reflect_pad(images).astype(F32)
    Bux = fir_up_matrix(hz)
    Bdx = fir_down_matrix(hz)
    up_taps = tap_structure(Bux, 2)
    dn_taps = tap_structure(Bdx, 1)
    # device down-x reads w2e[:, :, :, dm : dm+2H : 2] -> offsets must be >= 0
    assert min(d for d, _ in dn_taps[0]) >= 0

    all_tiles = [plan_image(A[i]) for i in range(N)]
    WRM, WCM = window_extents(all_tiles)
    WRF, WCF = finalize_tiles(all_tiles, WRM, WCM)
    assert WRF * WCF <= 32000, (WRF, WCF)  # quad idx = 2*(r*WCF+c) must fit u16

    # pack per-core inputs
    buy_pack = np.zeros((3, 128, UH), F32)
    buy_pack.reshape(384, UH)[:P] = Bux
    bdy_pack = np.zeros((5, 128, H), F32)
    bdy_pack.reshape(640, H)[:WT] = Bdx

    in_maps = []
    for core in range(ncores):
        idx_arr = np.zeros((NB, 128, SW), np.uint16)
        scal_arr = np.zeros((NB, 6, 128), F32)
        offb_arr = np.zeros((NB * 8,), np.int32)
        for b in range(NB):
            img, tyy = b // GY, b % GY
            gi = core * per + img
            tiles = all_tiles[gi]
            for g in range(8):
                t = tiles[tyy * GX + g]
                idx_arr[b, 16 * g:16 * g + 16, 0:SW] = wrap16(t["idxA"])
                for k in range(6):
                    scal_arr[b, k, 16 * g:16 * g + 16] = t["consts"][k]
                offb_arr[b * 8 + g] = img * 3 * UH * UW + t["r0"] * UW + t["c0"]
        in_maps.append({
            "xpad": np.ascontiguousarray(xpad[core * per:(core + 1) * per]),
            "buy": buy_pack, "bdy": bdy_pack,
            "idx": idx_arr, "scal": scal_arr, "offb": offb_arr,
        })

    nc = build_graph(WRF, WCF, WRM, WCM, up_taps, dn_taps)
    res = bass_utils.run_bass_kernel_spmd(nc, in_maps, core_ids=list(range(ncores)))
    out = np.concatenate([res.results[i]["out"] for i in range(ncores)], 0)
    kernel.last_results = res
    return out



# revision 50
# speedup vs baseline: 1.0275x; 1.0275x over previous
"""AdaAugment Trainium2 kernel: reflect-pad + FIR up2 + affine bilinear warp + FIR down2.

Self-contained. Strategy (per NeuronCore, data-parallel over batch: 4 images/core):
 - host: reflect pad, banded FIR matrices, per-tile warp plans (indices/weights consts)
 - device: x-FIRs on DVE (strided taps), y-FIRs on PE (banded matmuls),
   warp via gpsimd indirect_copy gathers over DMA'd U windows, bilinear lerp on DVE,
   weights from iota + per-partition affine (bit-exact host mirror).
One SPMD graph for all 8 cores; all per-image geometry is input data.
"""
import sys, os
for p in ("/opt/trn_rl_repo", "/root/.axon_site/_ro/trn_rl_repo"):
    if os.path.isdir(p) and p not in sys.path:
        sys.path.insert(0, p)
import numpy as np

F32 = np.float32
H = W = 256
FW = 12
HZ_PAD = 3
MARGIN = 38
P = H + 2 * MARGIN            # 332
UH = UW = 664
WT = (H + 2 * HZ_PAD) * 2     # 524
TY, TX = 32, 66
GY, GX = 17, 8
WTY, WTX = GY * TY, GX * TX   # 544, 528
NIDX = TY * TX                # 2112
SW = NIDX // 16               # 132 wrapped idx cols
NB = 4 * GY                   # 68 batches per core
ZOFF = F32(1024.0)
NIMG = 4                      # images per core


# ---------------- host planning (mirrors device fp32 ops bit-exactly) --------

def affine_params(theta, log_s, tx, ty):
    N = theta.shape[0]
    s = np.exp(log_s).astype(F32)
    c, sn = np.cos(theta).astype(F32), np.sin(theta).astype(F32)
    A = np.zeros((N, 2, 3), F32)
    for i in range(N):
        rot = np.array([[c[i], sn[i], 0], [-sn[i], c[i], 0], [0, 0, 1]], F32)
        scl = np.array([[1 / s[i], 0, 0], [0, 1 / s[i], 0], [0, 0, 1]], F32)
        trn = np.array([[1, 0, -tx[i] * W], [0, 1, -ty[i] * H], [0, 0, 1]], F32)
        g = (scl @ rot @ trn).astype(F32)
        g = (np.array([[2, 0, 0], [0, 2, 0], [0, 0, 1]], F32) @ g
             @ np.array([[.5, 0, 0], [0, .5, 0], [0, 0, 1]], F32)).astype(F32)
        g = (np.array([[1, 0, -.5], [0, 1, -.5], [0, 0, 1]], F32) @ g
             @ np.array([[1, 0, .5], [0, 1, .5], [0, 0, 1]], F32)).astype(F32)
        g = (np.array([[2.0 / UW, 0, 0], [0, 2.0 / UH, 0], [0, 0, 1]], F32) @ g
             @ np.array([[WT / 2.0, 0, 0], [0, WT / 2.0, 0], [0, 0, 1]], F32)).astype(F32)
        A[i] = g[:2, :]
    return A


def pixel_affine(A):
    Ad = A.astype(np.float64)
    ax = Ad[0, 0] * UW / WT
    bx = Ad[0, 1] * UW / WT
    cx = (Ad[0, 0] * (1.0 / WT - 1.0) + Ad[0, 1] * (1.0 / WT - 1.0) + Ad[0, 2] + 1.0) * UW / 2.0 - 0.5
    ay = Ad[1, 0] * UW / WT
    by = Ad[1, 1] * UW / WT
    cy = (Ad[1, 0] * (1.0 / WT - 1.0) + Ad[1, 1] * (1.0 / WT - 1.0) + Ad[1, 2] + 1.0) * UH / 2.0 - 0.5
    return F32(ax), F32(bx), F32(cx), F32(ay), F32(by), F32(cy)


def fir_up_matrix(f):
    f2 = np.asarray(f, np.float64) * 2.0
    p0 = (FW + 1) // 2
    B = np.zeros((P, 2 * P), np.float64)
    for m in range(P):
        jlo, jhi = p0 + 2 * m - (FW - 1), p0 + 2 * m
        for j in range(max(jlo, 0), min(jhi + 1, 2 * P)):
            B[m, j] = f2[FW - 1 - (p0 + 2 * m - j)]
    return B.astype(F32)


def fir_down_matrix(f):
    fd = np.asarray(f, np.float64)
    B = np.zeros((WT, H), np.float64)
    for j in range(H):
        for t in range(FW):
            m = 2 * j + 1 + t
            if 0 <= m < WT:
                B[m, j] = fd[t]
    return B.astype(F32)


def reflect_pad(img):
    return np.pad(img, ((0, 0), (0, 0), (MARGIN, MARGIN), (MARGIN, MARGIN)), mode="reflect")


def plan_image(A):
    ax, bx, cx, ay, by, cy = pixel_affine(A)
    tiles = []
    ly = np.arange(TY, dtype=F32)[:, None]
    lx = np.arange(TX, dtype=F32)[None, :]
    for ty in range(GY):
        for tg in range(GX):
            yo0, xo0 = ty * TY, tg * TX
            Cx = F32(F32(F32(ax * xo0) + F32(bx * yo0)) + F32(cx + float(ZOFF)))
            Cy = F32(F32(F32(ay * xo0) + F32(by * yo0)) + F32(cy + float(ZOFF)))
            # device order: tA = f32(f32(ly*b) + C); z = f32(f32(lx*a) + tA)
            zx = np.float32(np.float32(lx * ax) + np.float32(np.float32(ly * bx) + Cx))
            zy = np.float32(np.float32(lx * ay) + np.float32(np.float32(ly * by) + Cy))
            wx = np.fmod(zx, F32(1.0))
            wy = np.fmod(zy, F32(1.0))
            ix0 = np.floor(zx).astype(np.int64) - int(ZOFF)
            iy0 = np.floor(zy).astype(np.int64) - int(ZOFF)
            tiles.append(dict(ty=ty, tg=tg, ix0=ix0, iy0=iy0, wx=wx, wy=wy,
                              consts=(ax, bx, Cx, ay, by, Cy)))
    return tiles


def window_extents(all_tiles):
    WRM = WCM = 8
    for tiles in all_tiles:
        for t in tiles:
            vx = (t["ix0"] >= -1) & (t["ix0"] <= UW - 1)
            vy = (t["iy0"] >= -1) & (t["iy0"] <= UH - 1)
            use = vx & vy
            if use.any():
                c0i = max(int(t["ix0"][use].min()), 0)
                c1i = min(int(t["ix0"][use].max()) + 1, UW - 1)
                r0i = max(int(t["iy0"][use].min()), 0)
                r1i = min(int(t["iy0"][use].max()) + 1, UH - 1)
                WRM = max(WRM, r1i - r0i + 1)
                WCM = max(WCM, c1i - c0i + 1)
                t["r0"], t["c0"] = r0i, c0i
            else:
                t["r0"], t["c0"] = 0, 0
    return WRM, WCM


def finalize_tiles(all_tiles, WRM, WCM):
    WRF, WCF = WRM + 4, WCM + 4
    for tiles in all_tiles:
        for t in tiles:
            r0 = min(t["r0"], UH - WRM)
            c0 = min(t["c0"], UW - WCM)
            t["r0"], t["c0"] = r0, c0
            ix0, iy0 = t["ix0"], t["iy0"]
            lc = ix0 - c0 + 2
            lr = iy0 - r0 + 2
            lc = np.where(ix0 < c0 - 1, 0, lc)
            lc = np.where(ix0 == c0 - 1, 1, lc)
            lc = np.where(ix0 > c0 + WCM - 1, WCF - 2, lc)
            lr = np.where(iy0 < r0 - 1, 0, lr)
            lr = np.where(iy0 == r0 - 1, 1, lr)
            lr = np.where(iy0 > r0 + WRM - 1, WRF - 2, lr)
            # quad index into the row-pair interleaved window (2 elems per slot)
            t["idxA"] = (2 * (lr * WCF + lc)).astype(np.uint16).ravel()
    return WRF, WCF


def wrap16(idx_flat):
    """Wrap the full index stream into [16, n/16] (single indirect_copy call)."""
    return idx_flat.reshape(idx_flat.shape[0] // 16, 16).T


def tap_structure(B, up):
    """Extract (offsets, coeffs) per output parity from a banded FIR matrix.
    up=2: out col j=2q+par taps rows q+dm; up=1(down): out col j taps rows 2j+dm."""
    taps = []
    if up == 2:
        for par in (0, 1):
            q0 = B.shape[0] // 2
            col = B[:, 2 * q0 + par]
            rows = np.nonzero(col)[0]
            taps.append([(int(r - q0), float(col[r])) for r in rows])
    else:
        j0 = B.shape[1] // 2
        col = B[:, j0]
        rows = np.nonzero(col)[0]
        taps.append([(int(r - 2 * j0), float(col[r])) for r in rows])
    return taps


# ---------------- device graph ----------------------------------------------

def build_graph(WRF, WCF, WRM, WCM, up_taps, dn_taps):
    import concourse.bass as bass
    import concourse.bacc as bacc
    import concourse.mybir as mybir
    from concourse.tile import TileContext

    dt = mybir.dt
    ALU = mybir.AluOpType
    ACTF = mybir.ActivationFunctionType
    FL = dt.float32
    BF = dt.bfloat16

    nc = bacc.Bacc("TRN2", target_bir_lowering=False, debug=False, num_devices=8)
    nc.disable_value_cache = True
    xpad_t = nc.dram_tensor("xpad", [NIMG, 3, P, P], FL, kind="ExternalInput")
    buy_t = nc.dram_tensor("buy", [3, 128, UH], FL, kind="ExternalInput")
    bdy_t = nc.dram_tensor("bdy", [5, 128, H], FL, kind="ExternalInput")
    idx_t = nc.dram_tensor("idx", [NB, 128, SW], dt.uint16, kind="ExternalInput")
    scal_t = nc.dram_tensor("scal", [NB, 6, 128], FL, kind="ExternalInput")
    offb_t = nc.dram_tensor("offb", [NB * 8], dt.int32, kind="ExternalInput")
    dbg = "ExternalOutput" if os.environ.get("ADA_DEBUG") == "1" else "Internal"
    u_dram = nc.dram_tensor("u_dbg", [NIMG * 3 * UH * UW + UH * UW], BF, kind=dbg)
    w2_dram = nc.dram_tensor("w2_dbg", [NIMG * 3 * WTY * WTX], BF, kind=dbg)
    out_t = nc.dram_tensor("out", [NIMG, 3, H, W], FL, kind="ExternalOutput")

    def dap(th, offset, dims):
        return bass.AP(th, int(offset), [list(d) for d in dims])

    with TileContext(nc) as tc:
        with tc.tile_pool(name="const", bufs=1) as cpool, \
             tc.tile_pool(name="psum", bufs=4, space="PSUM") as ppool:

            # ---- constants staged once ----
            scal_sb = cpool.tile([128, NB, 6], FL, tag="scal")
            nc.sync.dma_start(out=scal_sb[:, :, :], in_=dap(
                scal_t, 0, [(1, 128), (6 * 128, NB), (128, 6)]))
            iota_xf = cpool.tile([128, NIDX], BF, tag="iotaxf")
            iota_yf = cpool.tile([128, NIDX], BF, tag="iotayf")

            # single window staging buffer (guards zeroed once)
            wb0 = cpool.tile([128, WRF * WCF], BF, tag="wb0")
            nc.vector.memset(wb0[:, :], 0.0)
            wb_list = [wb0, wb0]
            # row-pair interleaved windows (double-buffered): wb2[2j]=wb[j],
            # wb2[2j+1]=wb[j+WCF] so one gather index fetches all 4 corners
            wb2a = cpool.tile([128, 2 * WRF * WCF], BF, tag="wb2a")
            nc.vector.memset(wb2a[:, :], 0.0)
            wb2b = cpool.tile([128, 2 * WRF * WCF], BF, tag="wb2b")
            nc.vector.memset(wb2b[:, :], 0.0)
            wb2_list = [wb2a, wb2b]

            # =================== phase 1: FIR up (per image) ===================
            fir_pool_ctx = tc.tile_pool(name="fir", bufs=1)
            fpool = fir_pool_ctx.__enter__()
            buy_sb = fpool.tile([128, 3, UH], FL, tag="buy")
            nc.sync.dma_start(out=buy_sb[:, :, :], in_=dap(
                buy_t, 0, [(UH, 128), (128 * UH, 3), (1, UH)]))
            iota_xi = fpool.tile([128, NIDX], dt.int32, tag="iotai")
            nc.gpsimd.iota(iota_xi[:, :], pattern=[[0, TY], [1, TX]], base=0,
                           channel_multiplier=0)
            nc.scalar.copy(out=iota_xf[:, :], in_=iota_xi[:, :])
            nc.gpsimd.iota(iota_xi[:, :], pattern=[[1, TY], [0, TX]], base=0,
                           channel_multiplier=0)
            nc.scalar.copy(out=iota_yf[:, :], in_=iota_xi[:, :])
            for img in range(NIMG):
                xpe = fpool.tile([128, 3, 3, P + 12], FL, tag="xpe")
                nc.vector.memset(xpe[:, :, :, :], 0.0)
                # load 332 rows into (blk, part): blk 0-1 full, blk 2 rows 0-75
                for blk in range(3):
                    pr = 128 if blk < 2 else P - 256
                    nc.sync.dma_start(
                        out=xpe[0:pr, blk, :, 6:6 + P],
                        in_=dap(xpad_t, img * 3 * P * P + blk * 128 * P,
                                [(P, pr), (P * P, 3), (1, P)]))
                # up-x on DVE: T1[.., par::2] = sum taps
                t1 = fpool.tile([128, 3, 3, UH], FL, tag="t1")
                for par in (0, 1):
                    for k, (dm, cf) in enumerate(up_taps[par]):
                        src = xpe[:, :, :, 6 + dm:6 + dm + P]
                        dst = t1[:, :, :, par::2]
                        if k == 0:
                            nc.vector.tensor_scalar(dst, src, float(cf), None, ALU.mult)
                        else:
                            nc.vector.scalar_tensor_tensor(
                                dst, src, float(cf), dst, ALU.mult, ALU.add)
                # up-y on PE: per M-tile, accumulate over K partition-blocks
                for mt in range(6):
                    ms, me = mt * 128, min(mt * 128 + 128, UH)
                    mm = me - ms
                    # K-window rows from Buy sparsity: out col j taps rows (j-par)/2+dm
                    r_lo = max(ms // 2 + min(d for d, _ in up_taps[0] + up_taps[1]), 0)
                    r_hi = min((me - 1) // 2 + max(d for d, _ in up_taps[0] + up_taps[1]), P - 1)
                    blks = list(range(r_lo // 128, r_hi // 128 + 1))
                    for ch in range(3):
                        for cs in (0, 512):
                            ce = min(cs + 512, UH)
                            nn = ce - cs
                            ps = ppool.tile([128, 512], FL, tag="ps_u")
                            for bi, b in enumerate(blks):
                                nc.tensor.matmul(
                                    ps[0:mm, 0:nn],
                                    buy_sb[:, b, ms:me],
                                    t1[:, b, ch, cs:ce],
                                    start=(bi == 0), stop=(bi == len(blks) - 1))
                            # evac + store (cast to bf16)
                            usb = fpool.tile([128, 512], BF, tag="usb")
                            nc.scalar.copy(out=usb[0:mm, 0:nn], in_=ps[0:mm, 0:nn])
                            nc.sync.dma_start(
                                out=dap(u_dram,
                                        (img * 3 + ch) * UH * UW + ms * UW + cs,
                                        [(UW, mm), (1, nn)]),
                                in_=usb[0:mm, 0:nn])

            fir_pool_ctx.__exit__(None, None, None)
            # =================== phase 2: warp (68 batches) ===================
            warp_pool_ctx = tc.tile_pool(name="warp", bufs=1)
            wpool = warp_pool_ctx.__enter__()
            prev_wdmas = []
            state = {}

            def head(b, tc=tc):
                # stage idx/offsets, fetch windows, build interleaved wb2.
                # Emitted one batch ahead so it overlaps batch b-1's gathers;
                # high_priority pulls it earlier in the scheduler's ordering.
                ctx = tc.high_priority(offset=80)
                ctx.__enter__()
                wb = wb_list[b % 2]
                idx_sb = wpool.tile([128, SW], dt.uint16, tag="idx", bufs=2)
                nc.scalar.dma_start(out=idx_sb[:, :], in_=dap(
                    idx_t, b * 128 * SW, [(SW, 128), (1, SW)]))
                offb_b = wpool.tile([128, 8], dt.int32, tag="offb_b", bufs=2)
                ob_dma = nc.scalar.dma_start(out=offb_b[0:1, :], in_=dap(
                    offb_t, b * 8, [(8, 1), (1, 8)]))
                if prev_wdmas:
                    bass._add_dep_helper(
                        ob_dma.ins, prev_wdmas[-1].ins, sync=True,
                        reason="offb slot reuse waits past prior register loads")
                u_ap = u_dram.ap()
                lds, vals = nc.values_load_multi_w_load_instructions(
                    offb_b[0:1, 0:8], engines=[mybir.EngineType.Activation],
                    min_val=0, max_val=(NIMG - 1) * 3 * UH * UW + UH * UW,
                    skip_runtime_bounds_check=True)
                if prev_wdmas:
                    for ld in lds:
                        bass._add_dep_helper(
                            ld.ins, prev_wdmas[-1].ins, sync=False,
                            reason="bound window-offset register liveness")
                wdmas = []
                for g in range(8):
                    src = u_ap[bass.ds(vals[g], 3 * UH * UW)].rearrange(
                        "(c r x) -> c r x", c=3, x=UW)[:, 0:WRM, 0:WCM]
                    wbv = wb[16 * g:16 * g + 3, :].rearrange(
                        "p (r c) -> p r c", c=WCF)[:, 2:2 + WRM, 2:2 + WCM]
                    wdmas.append(nc.scalar.dma_start(out=wbv, in_=src))
                # interleave: even elems <- wb[j] (ACT), odd <- wb[j+WCF] (DVE)
                wb2 = wb2_list[b % 2]
                wb2v = wb2[:, :].rearrange("p (a b) -> p a b", b=2)
                NW = WRF * WCF
                bld_e = nc.scalar.copy(
                    out=wb2v[:, :, 0:1].rearrange("p a b -> p (a b)"), in_=wb[:, :])
                bld_o = nc.vector.tensor_scalar(
                    wb2v[:, 0:NW - WCF, 1:2].rearrange("p a b -> p (a b)"),
                    wb[:, WCF:], 1.0, None, ALU.mult)
                state[b] = (idx_sb, wb2)
                ctx.__exit__(None, None, None)
                return wdmas, bld_e, bld_o

            prev_wdmas, _, _ = head(0)
            for b in range(NB):
                img, ty = b // GY, b % GY
                # force next batch's wb2 build to schedule before this batch's
                # ACT/DVE compute so it hides under this batch's gathers
                if b + 1 < NB:
                    prev_wdmas, bld_e, bld_o = head(b + 1)
                    force_after = (bld_e, bld_o)
                else:
                    force_after = (None, None)
                idx_sb, wb2 = state.pop(b)
                # weights: zs = ax*iotaX + (bx*iotaY + Cx); w = frac(zs)
                tBb = wpool.tile([128, 2, NIDX], FL, tag="tB")
                tA = wpool.tile([128, NIDX], FL, tag="tA")
                for k, (o_a, o_b, o_c) in enumerate(((0, 1, 2), (3, 4, 5))):
                    # tA = b*iotaY + C        (ACT)
                    w_act = nc.scalar.activation(
                        tA[:, :], iota_yf[:, :], ACTF.Identity,
                        bias=scal_sb[:, b, o_c:o_c + 1],
                        scale=scal_sb[:, b, o_b:o_b + 1])
                    if False:
                        pass
                    # zs = (iotaX * a) + tA   (DVE stt)
                    nc.vector.scalar_tensor_tensor(
                        tBb[:, k, :], iota_xf[:, :], scal_sb[:, b, o_a:o_a + 1],
                        tA[:, :], ALU.mult, ALU.add)
                    # zf in tA: int-cast then float-cast in place
                    tAi = tA[:, :].bitcast(dt.int32)
                    nc.scalar.copy(out=tAi, in_=tBb[:, k, :])
                    nc.scalar.copy(out=tA[:, :], in_=tAi)
                    # fr = zs - zf (in place)
                    nc.vector.tensor_tensor(tBb[:, k, :], tBb[:, k, :], tA[:, :],
                                            ALU.subtract)
                # merged fixup for both weights: w = (fr<0) + fr  -> bf16
                wv = wpool.tile([128, 2, NIDX], BF, tag="wv")
                nc.vector.scalar_tensor_tensor(
                    wv[:, :, :], tBb[:, :, :], 0.0, tBb[:, :, :], ALU.is_lt, ALU.add)
                wx_t = wv[:, 0, :]
                wy_t = wv[:, 1, :]
                # quad gathers: one idx -> (v00,v10,v01,v11); dst cap 1024 elems
                gq = wpool.tile([128, NIDX, 4], BF, tag="gq", bufs=2)
                wb2d = wb2[:, :].rearrange("p (a b) -> p a b", b=4)
                for c0 in range(0, NIDX, 256):
                    c1 = min(c0 + 256, NIDX)
                    nc.gpsimd.indirect_copy(
                        gq[:, c0:c1, :], wb2d, idx_sb[:, c0 // 16:c1 // 16],
                        True)

                def ev(t, k):
                    return t[:, :, k:k + 1].rearrange("p a b -> p (a b)")
                # y-lerp both columns, then x-lerp
                tmp0 = tBb[:, 0, :]
                tmp1 = tBb[:, 1, :]
                l0 = nc.vector.tensor_tensor(tmp0, ev(gq, 1), ev(gq, 0),
                                             ALU.subtract)
                if force_after[1] is not None:
                    bass._add_dep_helper(
                        l0.ins, force_after[1].ins, sync=False,
                        reason="schedule next-batch wb2 build first on DVE")
                nc.vector.tensor_tensor(tmp0, tmp0, wy_t, ALU.mult)
                nc.vector.tensor_tensor(tmp0, tmp0, ev(gq, 0), ALU.add)
                nc.vector.tensor_tensor(tmp1, ev(gq, 3), ev(gq, 2), ALU.subtract)
                nc.vector.tensor_tensor(tmp1, tmp1, wy_t, ALU.mult)
                nc.vector.tensor_tensor(tmp1, tmp1, ev(gq, 2), ALU.add)
                nc.vector.tensor_tensor(tmp1, tmp1, tmp0, ALU.subtract)
                nc.vector.tensor_tensor(tmp1, tmp1, wx_t, ALU.mult)
                outt = wpool.tile([128, NIDX], BF, tag="outt", bufs=2)
                nc.vector.tensor_tensor(outt[:, :], tmp1, tmp0, ALU.add)
                # store stripe: one DMA per channel covering all 8 groups
                ov = outt[:, :].rearrange("(g s) (y x) -> g s y x", s=16, x=TX)
                for ch in range(3):
                    nc.sync.dma_start(
                        out=dap(w2_dram,
                                (img * 3 + ch) * WTY * WTX + ty * TY * WTX,
                                [(66, 8), (WTX, TY), (1, TX)]),
                        in_=ov[:, ch, :, :])

            warp_pool_ctx.__exit__(None, None, None)
            # =================== phase 3: FIR down (per image) =================
            dn_pool_ctx = tc.tile_pool(name="down", bufs=2)
            fpool = dn_pool_ctx.__enter__()
            bdy_sb = fpool.tile([128, 5, H], FL, tag="bdy")
            nc.sync.dma_start(out=bdy_sb[:, :, :], in_=dap(
                bdy_t, 0, [(H, 128), (128 * H, 5), (1, H)]))
            for img in range(NIMG):
                w2e = fpool.tile([128, 5, 3, WT], BF, tag="w2e")
                nc.vector.memset(w2e[:, :, :, :], 0.0)
                for blk in range(5):
                    pr = 128 if blk < 4 else WT - 512
                    nc.sync.dma_start(
                        out=w2e[0:pr, blk, :, :],
                        in_=dap(w2_dram, img * 3 * WTY * WTX + blk * 128 * WTX,
                                [(WTX, pr), (WTY * WTX, 3), (1, WT)]))
                # down-x on DVE (stride-2 taps)
                d1 = fpool.tile([128, 5, 3, H], FL, tag="d1")
                for k, (dm, cf) in enumerate(dn_taps[0]):
                    src = w2e[:, :, :, dm:dm + 2 * H:2]
                    if k == 0:
                        nc.vector.tensor_scalar(d1[:, :, :, :], src, float(cf), None, ALU.mult)
                    else:
                        nc.vector.scalar_tensor_tensor(
                            d1[:, :, :, :], src, float(cf), d1[:, :, :, :], ALU.mult, ALU.add)
                # down-y on PE
                dlo = min(d for d, _ in dn_taps[0])
                dhi = max(d for d, _ in dn_taps[0])
                for mt in range(2):
                    ms, me = mt * 128, mt * 128 + 128
                    r_lo = max(2 * ms + dlo, 0)
                    r_hi = min(2 * (me - 1) + dhi, WT - 1)
                    blks = list(range(r_lo // 128, r_hi // 128 + 1))
                    for ch in range(3):
                        ps = ppool.tile([128, 512], FL, tag="ps_o")
                        for bi, bb in enumerate(blks):
                            nc.tensor.matmul(
                                ps[0:128, 0:H],
                                bdy_sb[:, bb, ms:me],
                                d1[:, bb, ch, :],
                                start=(bi == 0), stop=(bi == len(blks) - 1))
                        ob = fpool.tile([128, H], FL, tag="ob")
                        nc.scalar.copy(out=ob[:, :], in_=ps[:, 0:H])
                        nc.sync.dma_start(
                            out=dap(out_t, (img * 3 + ch) * H * W + ms * W,
                                    [(W, 128), (1, H)]),
                            in_=ob[:, :])
            dn_pool_ctx.__exit__(None, None, None)

    nc.compile()
    return nc


# ---------------- entry point ------------------------------------------------

def kernel(**inputs):
    from concourse import bass_utils

    images = np.asarray(inputs["images"], np.float32)
    theta = np.asarray(inputs["theta"], np.float32)
    log_s = np.asarray(inputs["log_s"], np.float32)
    tx = np.asarray(inputs["tx"], np.float32)
    ty = np.asarray(inputs["ty"], np.float32)
    hz = np.asarray(inputs["hz_geom"], np.float32)
    N = images.shape[0]
    ncores = 8
    per = N // ncores

    A = affine_params(theta, log_s, tx, ty)
    xpad = reflect_pad(images).astype(F32)
    Bux = fir_up_matrix(hz)
    Bdx = fir_down_matrix(hz)
    up_taps = tap_structure(Bux, 2)
    dn_taps = tap_structure(Bdx, 1)
    # device down-x reads w2e[:, :, :, dm : dm+2H : 2] -> offsets must be >= 0
    assert min(d for d, _ in dn_taps[0]) >= 0

    all_tiles = [plan_image(A[i]) for i in range(N)]
    WRM, WCM = window_extents(all_tiles)
    WRF, WCF = finalize_tiles(all_tiles, WRM, WCM)
    assert WRF * WCF <= 32000, (WRF, WCF)  # quad idx = 2*(r*WCF+c) must fit u16

    # pack per-core inputs
    buy_pack = np.zeros((3, 128, UH), F32)
    buy_pack.reshape(384, UH)[:P] = Bux
    bdy_pack = np.zeros((5, 128, H), F32)
    bdy_pack.reshape(640, H)[:WT] = Bdx

    in_maps = []
    for core in range(ncores):
        idx_arr = np.zeros((NB, 128, SW), np.uint16)
        scal_arr = np.zeros((NB, 6, 128), F32)
        offb_arr = np.zeros((NB * 8,), np.int32)
        for b in range(NB):
            img, tyy = b // GY, b % GY
            gi = core * per + img
            tiles = all_tiles[gi]
            for g in range(8):
                t = tiles[tyy * GX + g]
                idx_arr[b, 16 * g:16 * g + 16, 0:SW] = wrap16(t["idxA"])
                for k in range(6):
                    scal_arr[b, k, 16 * g:16 * g + 16] = t["consts"][k]
                offb_arr[b * 8 + g] = img * 3 * UH * UW + t["r0"] * UW + t["c0"]
        in_maps.append({
            "xpad": np.ascontiguousarray(xpad[core * per:(core + 1) * per]),
            "buy": buy_pack, "bdy": bdy_pack,
            "idx": idx_arr, "scal": scal_arr, "offb": offb_arr,
        })

    nc = build_graph(WRF, WCF, WRM, WCM, up_taps, dn_taps)
    res = bass_utils.run_bass_kernel_spmd(nc, in_maps, core_ids=list(range(ncores)))
    out = np.concatenate([res.results[i]["out"] for i in range(ncores)], 0)
    kernel.last_results = res
    return out



# revision 51
# speedup vs baseline: 1.0373x; 1.0095x over previous
"""AdaAugment Trainium2 kernel: reflect-pad + FIR up2 + affine bilinear warp + FIR down2.

Self-contained. Strategy (per NeuronCore, data-parallel over batch: 4 images/core):
 - host: reflect pad, banded FIR matrices, per-tile warp plans (indices/weights consts)
 - device: x-FIRs on DVE (strided taps), y-FIRs on PE (banded matmuls),
   warp via gpsimd indirect_copy gathers over DMA'd U windows, bilinear lerp on DVE,
   weights from iota + per-partition affine (bit-exact host mirror).
One SPMD graph for all 8 cores; all per-image geometry is input data.
"""
import sys, os
for p in ("/opt/trn_rl_repo", "/root/.axon_site/_ro/trn_rl_repo"):
    if os.path.isdir(p) and p not in sys.path:
        sys.path.insert(0, p)
import numpy as np

F32 = np.float32
H = W = 256
FW = 12
HZ_PAD = 3
MARGIN = 38
P = H + 2 * MARGIN            # 332
UH = UW = 664
WT = (H + 2 * HZ_PAD) * 2     # 524
TY, TX = 32, 66
GY, GX = 17, 8
WTY, WTX = GY * TY, GX * TX   # 544, 528
NIDX = TY * TX                # 2112
SW = NIDX // 16               # 132 wrapped idx cols
NB = 4 * GY                   # 68 batches per core
ZOFF = F32(1024.0)
NIMG = 4                      # images per core


# ---------------- host planning (mirrors device fp32 ops bit-exactly) --------

def affine_params(theta, log_s, tx, ty):
    N = theta.shape[0]
    s = np.exp(log_s).astype(F32)
    c, sn = np.cos(theta).astype(F32), np.sin(theta).astype(F32)
    A = np.zeros((N, 2, 3), F32)
    for i in range(N):
        rot = np.array([[c[i], sn[i], 0], [-sn[i], c[i], 0], [0, 0, 1]], F32)
        scl = np.array([[1 / s[i], 0, 0], [0, 1 / s[i], 0], [0, 0, 1]], F32)
        trn = np.array([[1, 0, -tx[i] * W], [0, 1, -ty[i] * H], [0, 0, 1]], F32)
        g = (scl @ rot @ trn).astype(F32)
        g = (np.array([[2, 0, 0], [0, 2, 0], [0, 0, 1]], F32) @ g
             @ np.array([[.5, 0, 0], [0, .5, 0], [0, 0, 1]], F32)).astype(F32)
        g = (np.array([[1, 0, -.5], [0, 1, -.5], [0, 0, 1]], F32) @ g
             @ np.array([[1, 0, .5], [0, 1, .5], [0, 0, 1]], F32)).astype(F32)
        g = (np.array([[2.0 / UW, 0, 0], [0, 2.0 / UH, 0], [0, 0, 1]], F32) @ g
             @ np.array([[WT / 2.0, 0, 0], [0, WT / 2.0, 0], [0, 0, 1]], F32)).astype(F32)
        A[i] = g[:2, :]
    return A


def pixel_affine(A):
    Ad = A.astype(np.float64)
    ax = Ad[0, 0] * UW / WT
    bx = Ad[0, 1] * UW / WT
    cx = (Ad[0, 0] * (1.0 / WT - 1.0) + Ad[0, 1] * (1.0 / WT - 1.0) + Ad[0, 2] + 1.0) * UW / 2.0 - 0.5
    ay = Ad[1, 0] * UW / WT
    by = Ad[1, 1] * UW / WT
    cy = (Ad[1, 0] * (1.0 / WT - 1.0) + Ad[1, 1] * (1.0 / WT - 1.0) + Ad[1, 2] + 1.0) * UH / 2.0 - 0.5
    return F32(ax), F32(bx), F32(cx), F32(ay), F32(by), F32(cy)


def fir_up_matrix(f):
    f2 = np.asarray(f, np.float64) * 2.0
    p0 = (FW + 1) // 2
    B = np.zeros((P, 2 * P), np.float64)
    for m in range(P):
        jlo, jhi = p0 + 2 * m - (FW - 1), p0 + 2 * m
        for j in range(max(jlo, 0), min(jhi + 1, 2 * P)):
            B[m, j] = f2[FW - 1 - (p0 + 2 * m - j)]
    return B.astype(F32)


def fir_down_matrix(f):
    fd = np.asarray(f, np.float64)
    B = np.zeros((WT, H), np.float64)
    for j in range(H):
        for t in range(FW):
            m = 2 * j + 1 + t
            if 0 <= m < WT:
                B[m, j] = fd[t]
    return B.astype(F32)


def reflect_pad(img):
    return np.pad(img, ((0, 0), (0, 0), (MARGIN, MARGIN), (MARGIN, MARGIN)), mode="reflect")


def plan_image(A):
    ax, bx, cx, ay, by, cy = pixel_affine(A)
    tiles = []
    ly = np.arange(TY, dtype=F32)[:, None]
    lx = np.arange(TX, dtype=F32)[None, :]
    for ty in range(GY):
        for tg in range(GX):
            yo0, xo0 = ty * TY, tg * TX
            Cx = F32(F32(F32(ax * xo0) + F32(bx * yo0)) + F32(cx + float(ZOFF)))
            Cy = F32(F32(F32(ay * xo0) + F32(by * yo0)) + F32(cy + float(ZOFF)))
            # device order: tA = f32(f32(ly*b) + C); z = f32(f32(lx*a) + tA)
            zx = np.float32(np.float32(lx * ax) + np.float32(np.float32(ly * bx) + Cx))
            zy = np.float32(np.float32(lx * ay) + np.float32(np.float32(ly * by) + Cy))
            wx = np.fmod(zx, F32(1.0))
            wy = np.fmod(zy, F32(1.0))
            ix0 = np.floor(zx).astype(np.int64) - int(ZOFF)
            iy0 = np.floor(zy).astype(np.int64) - int(ZOFF)
            tiles.append(dict(ty=ty, tg=tg, ix0=ix0, iy0=iy0, wx=wx, wy=wy,
                              consts=(ax, bx, Cx, ay, by, Cy)))
    return tiles


def window_extents(all_tiles):
    WRM = WCM = 8
    for tiles in all_tiles:
        for t in tiles:
            vx = (t["ix0"] >= -1) & (t["ix0"] <= UW - 1)
            vy = (t["iy0"] >= -1) & (t["iy0"] <= UH - 1)
            use = vx & vy
            if use.any():
                c0i = max(int(t["ix0"][use].min()), 0)
                c1i = min(int(t["ix0"][use].max()) + 1, UW - 1)
                r0i = max(int(t["iy0"][use].min()), 0)
                r1i = min(int(t["iy0"][use].max()) + 1, UH - 1)
                WRM = max(WRM, r1i - r0i + 1)
                WCM = max(WCM, c1i - c0i + 1)
                t["r0"], t["c0"] = r0i, c0i
            else:
                t["r0"], t["c0"] = 0, 0
    return WRM, WCM


def finalize_tiles(all_tiles, WRM, WCM):
    WRF, WCF = WRM + 4, WCM + 4
    for tiles in all_tiles:
        for t in tiles:
            r0 = min(t["r0"], UH - WRM)
            c0 = min(t["c0"], UW - WCM)
            t["r0"], t["c0"] = r0, c0
            ix0, iy0 = t["ix0"], t["iy0"]
            lc = ix0 - c0 + 2
            lr = iy0 - r0 + 2
            lc = np.where(ix0 < c0 - 1, 0, lc)
            lc = np.where(ix0 == c0 - 1, 1, lc)
            lc = np.where(ix0 > c0 + WCM - 1, WCF - 2, lc)
            lr = np.where(iy0 < r0 - 1, 0, lr)
            lr = np.where(iy0 == r0 - 1, 1, lr)
            lr = np.where(iy0 > r0 + WRM - 1, WRF - 2, lr)
            # quad index into the row-pair interleaved window (2 elems per slot)
            t["idxA"] = (2 * (lr * WCF + lc)).astype(np.uint16).ravel()
    return WRF, WCF


def wrap16(idx_flat):
    """Wrap the full index stream into [16, n/16] (single indirect_copy call)."""
    return idx_flat.reshape(idx_flat.shape[0] // 16, 16).T


def tap_structure(B, up):
    """Extract (offsets, coeffs) per output parity from a banded FIR matrix.
    up=2: out col j=2q+par taps rows q+dm; up=1(down): out col j taps rows 2j+dm."""
    taps = []
    if up == 2:
        for par in (0, 1):
            q0 = B.shape[0] // 2
            col = B[:, 2 * q0 + par]
            rows = np.nonzero(col)[0]
            taps.append([(int(r - q0), float(col[r])) for r in rows])
    else:
        j0 = B.shape[1] // 2
        col = B[:, j0]
        rows = np.nonzero(col)[0]
        taps.append([(int(r - 2 * j0), float(col[r])) for r in rows])
    return taps


# ---------------- device graph ----------------------------------------------

def build_graph(WRF, WCF, WRM, WCM, up_taps, dn_taps):
    import concourse.bass as bass
    import concourse.bacc as bacc
    import concourse.mybir as mybir
    from concourse.tile import TileContext

    dt = mybir.dt
    ALU = mybir.AluOpType
    ACTF = mybir.ActivationFunctionType
    FL = dt.float32
    BF = dt.bfloat16

    nc = bacc.Bacc("TRN2", target_bir_lowering=False, debug=False, num_devices=8)
    nc.disable_value_cache = True
    xpad_t = nc.dram_tensor("xpad", [NIMG, 3, P, P], FL, kind="ExternalInput")
    buy_t = nc.dram_tensor("buy", [3, 128, UH], FL, kind="ExternalInput")
    bdy_t = nc.dram_tensor("bdy", [5, 128, H], FL, kind="ExternalInput")
    idx_t = nc.dram_tensor("idx", [NB, 128, SW], dt.uint16, kind="ExternalInput")
    scal_t = nc.dram_tensor("scal", [NB, 6, 128], FL, kind="ExternalInput")
    offb_t = nc.dram_tensor("offb", [NB * 8], dt.int32, kind="ExternalInput")
    dbg = "ExternalOutput" if os.environ.get("ADA_DEBUG") == "1" else "Internal"
    u_dram = nc.dram_tensor("u_dbg", [NIMG * 3 * UH * UW + UH * UW], BF, kind=dbg)
    w2_dram = nc.dram_tensor("w2_dbg", [NIMG * 3 * WTY * WTX], BF, kind=dbg)
    out_t = nc.dram_tensor("out", [NIMG, 3, H, W], FL, kind="ExternalOutput")

    def dap(th, offset, dims):
        return bass.AP(th, int(offset), [list(d) for d in dims])

    with TileContext(nc) as tc:
        with tc.tile_pool(name="const", bufs=1) as cpool, \
             tc.tile_pool(name="psum", bufs=4, space="PSUM") as ppool:

            # ---- constants staged once ----
            scal_sb = cpool.tile([128, NB, 6], FL, tag="scal")
            nc.sync.dma_start(out=scal_sb[:, :, :], in_=dap(
                scal_t, 0, [(1, 128), (6 * 128, NB), (128, 6)]))
            iota_xf = cpool.tile([128, NIDX], BF, tag="iotaxf")
            iota_yf = cpool.tile([128, NIDX], BF, tag="iotayf")

            # single window staging buffer (guards zeroed once)
            wb0 = cpool.tile([128, WRF * WCF], BF, tag="wb0")
            nc.vector.memset(wb0[:, :], 0.0)
            wb_list = [wb0, wb0]
            # row-pair interleaved windows (double-buffered): wb2[2j]=wb[j],
            # wb2[2j+1]=wb[j+WCF] so one gather index fetches all 4 corners
            wb2a = cpool.tile([128, 2 * WRF * WCF], BF, tag="wb2a")
            nc.vector.memset(wb2a[:, :], 0.0)
            wb2b = cpool.tile([128, 2 * WRF * WCF], BF, tag="wb2b")
            nc.vector.memset(wb2b[:, :], 0.0)
            wb2_list = [wb2a, wb2b]

            # =================== phase 1: FIR up (per image) ===================
            fir_pool_ctx = tc.tile_pool(name="fir", bufs=1)
            fpool = fir_pool_ctx.__enter__()
            buy_sb = fpool.tile([128, 3, UH], FL, tag="buy")
            nc.sync.dma_start(out=buy_sb[:, :, :], in_=dap(
                buy_t, 0, [(UH, 128), (128 * UH, 3), (1, UH)]))
            iota_xi = fpool.tile([128, NIDX], dt.int32, tag="iotai")
            nc.gpsimd.iota(iota_xi[:, :], pattern=[[0, TY], [1, TX]], base=0,
                           channel_multiplier=0)
            nc.scalar.copy(out=iota_xf[:, :], in_=iota_xi[:, :])
            nc.gpsimd.iota(iota_xi[:, :], pattern=[[1, TY], [0, TX]], base=0,
                           channel_multiplier=0)
            nc.scalar.copy(out=iota_yf[:, :], in_=iota_xi[:, :])
            for img in range(NIMG):
                xpe = fpool.tile([128, 3, 3, P + 12], FL, tag="xpe")
                nc.vector.memset(xpe[:, :, :, :], 0.0)
                # load 332 rows into (blk, part): blk 0-1 full, blk 2 rows 0-75
                for blk in range(3):
                    pr = 128 if blk < 2 else P - 256
                    nc.sync.dma_start(
                        out=xpe[0:pr, blk, :, 6:6 + P],
                        in_=dap(xpad_t, img * 3 * P * P + blk * 128 * P,
                                [(P, pr), (P * P, 3), (1, P)]))
                # up-x on DVE: T1[.., par::2] = sum taps
                t1 = fpool.tile([128, 3, 3, UH], FL, tag="t1", bufs=2)
                for par in (0, 1):
                    for k, (dm, cf) in enumerate(up_taps[par]):
                        src = xpe[:, :, :, 6 + dm:6 + dm + P]
                        dst = t1[:, :, :, par::2]
                        if k == 0:
                            nc.vector.tensor_scalar(dst, src, float(cf), None, ALU.mult)
                        else:
                            nc.vector.scalar_tensor_tensor(
                                dst, src, float(cf), dst, ALU.mult, ALU.add)
                # up-y on PE: per M-tile, accumulate over K partition-blocks
                for mt in range(6):
                    ms, me = mt * 128, min(mt * 128 + 128, UH)
                    mm = me - ms
                    # K-window rows from Buy sparsity: out col j taps rows (j-par)/2+dm
                    r_lo = max(ms // 2 + min(d for d, _ in up_taps[0] + up_taps[1]), 0)
                    r_hi = min((me - 1) // 2 + max(d for d, _ in up_taps[0] + up_taps[1]), P - 1)
                    blks = list(range(r_lo // 128, r_hi // 128 + 1))
                    for ch in range(3):
                        for cs in (0, 512):
                            ce = min(cs + 512, UH)
                            nn = ce - cs
                            ps = ppool.tile([128, 512], FL, tag="ps_u")
                            for bi, b in enumerate(blks):
                                nc.tensor.matmul(
                                    ps[0:mm, 0:nn],
                                    buy_sb[:, b, ms:me],
                                    t1[:, b, ch, cs:ce],
                                    start=(bi == 0), stop=(bi == len(blks) - 1))
                            # evac + store (cast to bf16)
                            usb = fpool.tile([128, 512], BF, tag="usb")
                            nc.scalar.copy(out=usb[0:mm, 0:nn], in_=ps[0:mm, 0:nn])
                            nc.sync.dma_start(
                                out=dap(u_dram,
                                        (img * 3 + ch) * UH * UW + ms * UW + cs,
                                        [(UW, mm), (1, nn)]),
                                in_=usb[0:mm, 0:nn])

            fir_pool_ctx.__exit__(None, None, None)
            # =================== phase 2: warp (68 batches) ===================
            warp_pool_ctx = tc.tile_pool(name="warp", bufs=1)
            wpool = warp_pool_ctx.__enter__()
            prev_wdmas = []
            state = {}

            def head(b, tc=tc):
                # stage idx/offsets, fetch windows, build interleaved wb2.
                # Emitted one batch ahead so it overlaps batch b-1's gathers;
                # high_priority pulls it earlier in the scheduler's ordering.
                ctx = tc.high_priority(offset=80)
                ctx.__enter__()
                wb = wb_list[b % 2]
                idx_sb = wpool.tile([128, SW], dt.uint16, tag="idx", bufs=2)
                nc.scalar.dma_start(out=idx_sb[:, :], in_=dap(
                    idx_t, b * 128 * SW, [(SW, 128), (1, SW)]))
                offb_b = wpool.tile([128, 8], dt.int32, tag="offb_b", bufs=2)
                ob_dma = nc.scalar.dma_start(out=offb_b[0:1, :], in_=dap(
                    offb_t, b * 8, [(8, 1), (1, 8)]))
                if prev_wdmas:
                    bass._add_dep_helper(
                        ob_dma.ins, prev_wdmas[-1].ins, sync=True,
                        reason="offb slot reuse waits past prior register loads")
                u_ap = u_dram.ap()
                lds, vals = nc.values_load_multi_w_load_instructions(
                    offb_b[0:1, 0:8], engines=[mybir.EngineType.Activation],
                    min_val=0, max_val=(NIMG - 1) * 3 * UH * UW + UH * UW,
                    skip_runtime_bounds_check=True)
                if prev_wdmas:
                    for ld in lds:
                        bass._add_dep_helper(
                            ld.ins, prev_wdmas[-1].ins, sync=False,
                            reason="bound window-offset register liveness")
                wdmas = []
                for g in range(8):
                    src = u_ap[bass.ds(vals[g], 3 * UH * UW)].rearrange(
                        "(c r x) -> c r x", c=3, x=UW)[:, 0:WRM, 0:WCM]
                    wbv = wb[16 * g:16 * g + 3, :].rearrange(
                        "p (r c) -> p r c", c=WCF)[:, 2:2 + WRM, 2:2 + WCM]
                    wdmas.append(nc.scalar.dma_start(out=wbv, in_=src))
                # interleave: even elems <- wb[j] (ACT), odd <- wb[j+WCF] (DVE)
                wb2 = wb2_list[b % 2]
                wb2v = wb2[:, :].rearrange("p (a b) -> p a b", b=2)
                NW = WRF * WCF
                bld_e = nc.scalar.copy(
                    out=wb2v[:, :, 0:1].rearrange("p a b -> p (a b)"), in_=wb[:, :])
                bld_o = nc.vector.tensor_scalar(
                    wb2v[:, 0:NW - WCF, 1:2].rearrange("p a b -> p (a b)"),
                    wb[:, WCF:], 1.0, None, ALU.mult)
                state[b] = (idx_sb, wb2)
                ctx.__exit__(None, None, None)
                return wdmas, bld_e, bld_o

            prev_wdmas, _, _ = head(0)
            for b in range(NB):
                img, ty = b // GY, b % GY
                # force next batch's wb2 build to schedule before this batch's
                # ACT/DVE compute so it hides under this batch's gathers
                if b + 1 < NB:
                    prev_wdmas, bld_e, bld_o = head(b + 1)
                    force_after = (bld_e, bld_o)
                else:
                    force_after = (None, None)
                idx_sb, wb2 = state.pop(b)
                # weights: zs = ax*iotaX + (bx*iotaY + Cx); w = frac(zs)
                tBb = wpool.tile([128, 2, NIDX], FL, tag="tB")
                tA = wpool.tile([128, NIDX], FL, tag="tA")
                for k, (o_a, o_b, o_c) in enumerate(((0, 1, 2), (3, 4, 5))):
                    # tA = b*iotaY + C        (ACT)
                    w_act = nc.scalar.activation(
                        tA[:, :], iota_yf[:, :], ACTF.Identity,
                        bias=scal_sb[:, b, o_c:o_c + 1],
                        scale=scal_sb[:, b, o_b:o_b + 1])
                    if False:
                        pass
                    # zs = (iotaX * a) + tA   (DVE stt)
                    nc.vector.scalar_tensor_tensor(
                        tBb[:, k, :], iota_xf[:, :], scal_sb[:, b, o_a:o_a + 1],
                        tA[:, :], ALU.mult, ALU.add)
                    # zf in tA: int-cast then float-cast in place
                    tAi = tA[:, :].bitcast(dt.int32)
                    nc.scalar.copy(out=tAi, in_=tBb[:, k, :])
                    nc.scalar.copy(out=tA[:, :], in_=tAi)
                    # fr = zs - zf (in place)
                    nc.vector.tensor_tensor(tBb[:, k, :], tBb[:, k, :], tA[:, :],
                                            ALU.subtract)
                # merged fixup for both weights: w = (fr<0) + fr  -> bf16
                wv = wpool.tile([128, 2, NIDX], BF, tag="wv")
                nc.vector.scalar_tensor_tensor(
                    wv[:, :, :], tBb[:, :, :], 0.0, tBb[:, :, :], ALU.is_lt, ALU.add)
                wx_t = wv[:, 0, :]
                wy_t = wv[:, 1, :]
                # quad gathers: one idx -> (v00,v10,v01,v11); dst cap 1024 elems
                gq = wpool.tile([128, NIDX, 4], BF, tag="gq", bufs=2)
                wb2d = wb2[:, :].rearrange("p (a b) -> p a b", b=4)
                for c0 in range(0, NIDX, 256):
                    c1 = min(c0 + 256, NIDX)
                    nc.gpsimd.indirect_copy(
                        gq[:, c0:c1, :], wb2d, idx_sb[:, c0 // 16:c1 // 16],
                        True)

                def ev(t, k):
                    return t[:, :, k:k + 1].rearrange("p a b -> p (a b)")
                # y-lerp both columns, then x-lerp
                tmp0 = tBb[:, 0, :]
                tmp1 = tBb[:, 1, :]
                l0 = nc.vector.tensor_tensor(tmp0, ev(gq, 1), ev(gq, 0),
                                             ALU.subtract)
                if force_after[1] is not None:
                    bass._add_dep_helper(
                        l0.ins, force_after[1].ins, sync=False,
                        reason="schedule next-batch wb2 build first on DVE")
                nc.vector.tensor_tensor(tmp0, tmp0, wy_t, ALU.mult)
                nc.vector.tensor_tensor(tmp0, tmp0, ev(gq, 0), ALU.add)
                nc.vector.tensor_tensor(tmp1, ev(gq, 3), ev(gq, 2), ALU.subtract)
                nc.vector.tensor_tensor(tmp1, tmp1, wy_t, ALU.mult)
                nc.vector.tensor_tensor(tmp1, tmp1, ev(gq, 2), ALU.add)
                nc.vector.tensor_tensor(tmp1, tmp1, tmp0, ALU.subtract)
                nc.vector.tensor_tensor(tmp1, tmp1, wx_t, ALU.mult)
                outt = wpool.tile([128, NIDX], BF, tag="outt", bufs=2)
                nc.vector.tensor_tensor(outt[:, :], tmp1, tmp0, ALU.add)
                # store stripe: one DMA per channel covering all 8 groups
                ov = outt[:, :].rearrange("(g s) (y x) -> g s y x", s=16, x=TX)
                for ch in range(3):
                    nc.sync.dma_start(
                        out=dap(w2_dram,
                                (img * 3 + ch) * WTY * WTX + ty * TY * WTX,
                                [(66, 8), (WTX, TY), (1, TX)]),
                        in_=ov[:, ch, :, :])

            warp_pool_ctx.__exit__(None, None, None)
            # =================== phase 3: FIR down (per image) =================
            dn_pool_ctx = tc.tile_pool(name="down", bufs=2)
            fpool = dn_pool_ctx.__enter__()
            bdy_sb = fpool.tile([128, 5, H], FL, tag="bdy")
            nc.sync.dma_start(out=bdy_sb[:, :, :], in_=dap(
                bdy_t, 0, [(H, 128), (128 * H, 5), (1, H)]))
            for img in range(NIMG):
                w2e = fpool.tile([128, 5, 3, WT], BF, tag="w2e")
                nc.vector.memset(w2e[:, :, :, :], 0.0)
                for blk in range(5):
                    pr = 128 if blk < 4 else WT - 512
                    nc.sync.dma_start(
                        out=w2e[0:pr, blk, :, :],
                        in_=dap(w2_dram, img * 3 * WTY * WTX + blk * 128 * WTX,
                                [(WTX, pr), (WTY * WTX, 3), (1, WT)]))
                # down-x on DVE (stride-2 taps)
                d1 = fpool.tile([128, 5, 3, H], FL, tag="d1")
                for k, (dm, cf) in enumerate(dn_taps[0]):
                    src = w2e[:, :, :, dm:dm + 2 * H:2]
                    if k == 0:
                        nc.vector.tensor_scalar(d1[:, :, :, :], src, float(cf), None, ALU.mult)
                    else:
                        nc.vector.scalar_tensor_tensor(
                            d1[:, :, :, :], src, float(cf), d1[:, :, :, :], ALU.mult, ALU.add)
                # down-y on PE
                dlo = min(d for d, _ in dn_taps[0])
                dhi = max(d for d, _ in dn_taps[0])
                for mt in range(2):
                    ms, me = mt * 128, mt * 128 + 128
                    r_lo = max(2 * ms + dlo, 0)
                    r_hi = min(2 * (me - 1) + dhi, WT - 1)
                    blks = list(range(r_lo // 128, r_hi // 128 + 1))
                    for ch in range(3):
                        ps = ppool.tile([128, 512], FL, tag="ps_o")
                        for bi, bb in enumerate(blks):
                            nc.tensor.matmul(
                                ps[0:128, 0:H],
                                bdy_sb[:, bb, ms:me],
                                d1[:, bb, ch, :],
                                start=(bi == 0), stop=(bi == len(blks) - 1))
                        ob = fpool.tile([128, H], FL, tag="ob")
                        nc.scalar.copy(out=ob[:, :], in_=ps[:, 0:H])
                        nc.sync.dma_start(
                            out=dap(out_t, (img * 3 + ch) * H * W + ms * W,
                                    [(W, 128), (1, H)]),
                            in_=ob[:, :])
            dn_pool_ctx.__exit__(None, None, None)

    nc.compile()
    return nc


# ---------------- entry point ------------------------------------------------

def kernel(**inputs):
    from concourse import bass_utils

    images = np.asarray(inputs["images"], np.float32)
    theta = np.asarray(inputs["theta"], np.float32)
    log_s = np.asarray(inputs["log_s"], np.float32)
    tx = np.asarray(inputs["tx"], np.float32)
    ty = np.asarray(inputs["ty"], np.float32)
    hz = np.asarray(inputs["hz_geom"], np.float32)
    N = images.shape[0]
    ncores = 8
    per = N // ncores

    A = affine_params(theta, log_s, tx, ty)
    xpad = reflect_pad(images).astype(F32)
    Bux = fir_up_matrix(hz)
    Bdx = fir_down_matrix(hz)
    up_taps = tap_structure(Bux, 2)
    dn_taps = tap_structure(Bdx, 1)
    # device down-x reads w2e[:, :, :, dm : dm+2H : 2] -> offsets must be >= 0
    assert min(d for d, _ in dn_taps[0]) >= 0

    all_tiles = [plan_image(A[i]) for i in range(N)]
    WRM, WCM = window_extents(all_tiles)
    WRF, WCF = finalize_tiles(all_tiles, WRM, WCM)
    assert WRF * WCF <= 32000, (WRF, WCF)  # quad idx = 2*(r*WCF+c) must fit u16

    # pack per-core inputs
    buy_pack = np.zeros((3, 128, UH), F32)
    buy_pack.reshape(384, UH)[:P] = Bux
    bdy_pack = np.zeros((5, 128, H), F32)
    bdy_pack.reshape(640, H)[:WT] = Bdx

    in_maps = []
    for core in range(ncores):
        idx_arr = np.zeros((NB, 128, SW), np.uint16)
        scal_arr = np.zeros((NB, 6, 128), F32)
        offb_arr = np.zeros((NB * 8,), np.int32)
        for b in range(NB):
            img, tyy = b // GY, b % GY
            gi = core * per + img
            tiles = all_tiles[gi]
            for g in range(8):
                t = tiles[tyy * GX + g]
                idx_arr[b, 16 * g:16 * g + 16, 0:SW] = wrap16(t["idxA"])
                for k in range(6):
                    scal_arr[b, k, 16 * g:16 * g + 16] = t["consts"][k]
                offb_arr[b * 8 + g] = img * 3 * UH * UW + t["r0"] * UW + t["c0"]
        in_maps.append({
            "xpad": np.ascontiguousarray(xpad[core * per:(core + 1) * per]),
            "buy": buy_pack, "bdy": bdy_pack,
            "idx": idx_arr, "scal": scal_arr, "offb": offb_arr,
        })

    nc = build_graph(WRF, WCF, WRM, WCM, up_taps, dn_taps)
    res = bass_utils.run_bass_kernel_spmd(nc, in_maps, core_ids=list(range(ncores)))
    out = np.concatenate([res.results[i]["out"] for i in range(ncores)], 0)
    kernel.last_results = res
    return out



# revision 52
# speedup vs baseline: 1.0566x; 1.0186x over previous
"""AdaAugment Trainium2 kernel: reflect-pad + FIR up2 + affine bilinear warp + FIR down2.

Self-contained. Strategy (per NeuronCore, data-parallel over batch: 4 images/core):
 - host: reflect pad, banded FIR matrices, per-tile warp plans (indices/weights consts)
 - device: x-FIRs on DVE (strided taps), y-FIRs on PE (banded matmuls),
   warp via gpsimd indirect_copy gathers over DMA'd U windows, bilinear lerp on DVE,
   weights from iota + per-partition affine (bit-exact host mirror).
One SPMD graph for all 8 cores; all per-image geometry is input data.
"""
import sys, os
for p in ("/opt/trn_rl_repo", "/root/.axon_site/_ro/trn_rl_repo"):
    if os.path.isdir(p) and p not in sys.path:
        sys.path.insert(0, p)
import numpy as np

F32 = np.float32
H = W = 256
FW = 12
HZ_PAD = 3
MARGIN = 38
P = H + 2 * MARGIN            # 332
UH = UW = 664
WT = (H + 2 * HZ_PAD) * 2     # 524
TY, TX = 32, 66
GY, GX = 17, 8
WTY, WTX = GY * TY, GX * TX   # 544, 528
NIDX = TY * TX                # 2112
SW = NIDX // 16               # 132 wrapped idx cols
NB = 4 * GY                   # 68 batches per core
ZOFF = F32(1024.0)
NIMG = 4                      # images per core


# ---------------- host planning (mirrors device fp32 ops bit-exactly) --------

def affine_params(theta, log_s, tx, ty):
    N = theta.shape[0]
    s = np.exp(log_s).astype(F32)
    c, sn = np.cos(theta).astype(F32), np.sin(theta).astype(F32)
    A = np.zeros((N, 2, 3), F32)
    for i in range(N):
        rot = np.array([[c[i], sn[i], 0], [-sn[i], c[i], 0], [0, 0, 1]], F32)
        scl = np.array([[1 / s[i], 0, 0], [0, 1 / s[i], 0], [0, 0, 1]], F32)
        trn = np.array([[1, 0, -tx[i] * W], [0, 1, -ty[i] * H], [0, 0, 1]], F32)
        g = (scl @ rot @ trn).astype(F32)
        g = (np.array([[2, 0, 0], [0, 2, 0], [0, 0, 1]], F32) @ g
             @ np.array([[.5, 0, 0], [0, .5, 0], [0, 0, 1]], F32)).astype(F32)
        g = (np.array([[1, 0, -.5], [0, 1, -.5], [0, 0, 1]], F32) @ g
             @ np.array([[1, 0, .5], [0, 1, .5], [0, 0, 1]], F32)).astype(F32)
        g = (np.array([[2.0 / UW, 0, 0], [0, 2.0 / UH, 0], [0, 0, 1]], F32) @ g
             @ np.array([[WT / 2.0, 0, 0], [0, WT / 2.0, 0], [0, 0, 1]], F32)).astype(F32)
        A[i] = g[:2, :]
    return A


def pixel_affine(A):
    Ad = A.astype(np.float64)
    ax = Ad[0, 0] * UW / WT
    bx = Ad[0, 1] * UW / WT
    cx = (Ad[0, 0] * (1.0 / WT - 1.0) + Ad[0, 1] * (1.0 / WT - 1.0) + Ad[0, 2] + 1.0) * UW / 2.0 - 0.5
    ay = Ad[1, 0] * UW / WT
    by = Ad[1, 1] * UW / WT
    cy = (Ad[1, 0] * (1.0 / WT - 1.0) + Ad[1, 1] * (1.0 / WT - 1.0) + Ad[1, 2] + 1.0) * UH / 2.0 - 0.5
    return F32(ax), F32(bx), F32(cx), F32(ay), F32(by), F32(cy)


def fir_up_matrix(f):
    f2 = np.asarray(f, np.float64) * 2.0
    p0 = (FW + 1) // 2
    B = np.zeros((P, 2 * P), np.float64)
    for m in range(P):
        jlo, jhi = p0 + 2 * m - (FW - 1), p0 + 2 * m
        for j in range(max(jlo, 0), min(jhi + 1, 2 * P)):
            B[m, j] = f2[FW - 1 - (p0 + 2 * m - j)]
    return B.astype(F32)


def fir_down_matrix(f):
    fd = np.asarray(f, np.float64)
    B = np.zeros((WT, H), np.float64)
    for j in range(H):
        for t in range(FW):
            m = 2 * j + 1 + t
            if 0 <= m < WT:
                B[m, j] = fd[t]
    return B.astype(F32)


def reflect_pad(img):
    return np.pad(img, ((0, 0), (0, 0), (MARGIN, MARGIN), (MARGIN, MARGIN)), mode="reflect")


def plan_image(A):
    ax, bx, cx, ay, by, cy = pixel_affine(A)
    tiles = []
    ly = np.arange(TY, dtype=F32)[:, None]
    lx = np.arange(TX, dtype=F32)[None, :]
    for ty in range(GY):
        for tg in range(GX):
            yo0, xo0 = ty * TY, tg * TX
            Cx = F32(F32(F32(ax * xo0) + F32(bx * yo0)) + F32(cx + float(ZOFF)))
            Cy = F32(F32(F32(ay * xo0) + F32(by * yo0)) + F32(cy + float(ZOFF)))
            # device order: tA = f32(f32(ly*b) + C); z = f32(f32(lx*a) + tA)
            zx = np.float32(np.float32(lx * ax) + np.float32(np.float32(ly * bx) + Cx))
            zy = np.float32(np.float32(lx * ay) + np.float32(np.float32(ly * by) + Cy))
            wx = np.fmod(zx, F32(1.0))
            wy = np.fmod(zy, F32(1.0))
            ix0 = np.floor(zx).astype(np.int64) - int(ZOFF)
            iy0 = np.floor(zy).astype(np.int64) - int(ZOFF)
            tiles.append(dict(ty=ty, tg=tg, ix0=ix0, iy0=iy0, wx=wx, wy=wy,
                              consts=(ax, bx, Cx, ay, by, Cy)))
    return tiles


def window_extents(all_tiles):
    WRM = WCM = 8
    for tiles in all_tiles:
        for t in tiles:
            vx = (t["ix0"] >= -1) & (t["ix0"] <= UW - 1)
            vy = (t["iy0"] >= -1) & (t["iy0"] <= UH - 1)
            use = vx & vy
            if use.any():
                c0i = max(int(t["ix0"][use].min()), 0)
                c1i = min(int(t["ix0"][use].max()) + 1, UW - 1)
                r0i = max(int(t["iy0"][use].min()), 0)
                r1i = min(int(t["iy0"][use].max()) + 1, UH - 1)
                WRM = max(WRM, r1i - r0i + 1)
                WCM = max(WCM, c1i - c0i + 1)
                t["r0"], t["c0"] = r0i, c0i
            else:
                t["r0"], t["c0"] = 0, 0
    return WRM, WCM


def finalize_tiles(all_tiles, WRM, WCM):
    WRF, WCF = WRM + 4, WCM + 4
    for tiles in all_tiles:
        for t in tiles:
            r0 = min(t["r0"], UH - WRM)
            c0 = min(t["c0"], UW - WCM)
            t["r0"], t["c0"] = r0, c0
            ix0, iy0 = t["ix0"], t["iy0"]
            lc = ix0 - c0 + 2
            lr = iy0 - r0 + 2
            lc = np.where(ix0 < c0 - 1, 0, lc)
            lc = np.where(ix0 == c0 - 1, 1, lc)
            lc = np.where(ix0 > c0 + WCM - 1, WCF - 2, lc)
            lr = np.where(iy0 < r0 - 1, 0, lr)
            lr = np.where(iy0 == r0 - 1, 1, lr)
            lr = np.where(iy0 > r0 + WRM - 1, WRF - 2, lr)
            # quad index into the row-pair interleaved window (2 elems per slot)
            t["idxA"] = (2 * (lr * WCF + lc)).astype(np.uint16).ravel()
    return WRF, WCF


def wrap16(idx_flat):
    """Wrap the full index stream into [16, n/16] (single indirect_copy call)."""
    return idx_flat.reshape(idx_flat.shape[0] // 16, 16).T


def tap_structure(B, up):
    """Extract (offsets, coeffs) per output parity from a banded FIR matrix.
    up=2: out col j=2q+par taps rows q+dm; up=1(down): out col j taps rows 2j+dm."""
    taps = []
    if up == 2:
        for par in (0, 1):
            q0 = B.shape[0] // 2
            col = B[:, 2 * q0 + par]
            rows = np.nonzero(col)[0]
            taps.append([(int(r - q0), float(col[r])) for r in rows])
    else:
        j0 = B.shape[1] // 2
        col = B[:, j0]
        rows = np.nonzero(col)[0]
        taps.append([(int(r - 2 * j0), float(col[r])) for r in rows])
    return taps


# ---------------- device graph ----------------------------------------------

def build_graph(WRF, WCF, WRM, WCM, up_taps, dn_taps):
    import concourse.bass as bass
    import concourse.bacc as bacc
    import concourse.mybir as mybir
    from concourse.tile import TileContext

    dt = mybir.dt
    ALU = mybir.AluOpType
    ACTF = mybir.ActivationFunctionType
    FL = dt.float32
    BF = dt.bfloat16

    nc = bacc.Bacc("TRN2", target_bir_lowering=False, debug=False, num_devices=8)
    nc.disable_value_cache = True
    xpad_t = nc.dram_tensor("xpad", [NIMG, 3, P, P], FL, kind="ExternalInput")
    buy_t = nc.dram_tensor("buy", [3, 128, UH], FL, kind="ExternalInput")
    bdy_t = nc.dram_tensor("bdy", [5, 128, H], FL, kind="ExternalInput")
    idx_t = nc.dram_tensor("idx", [NB, 128, SW], dt.uint16, kind="ExternalInput")
    scal_t = nc.dram_tensor("scal", [NB, 6, 128], FL, kind="ExternalInput")
    offb_t = nc.dram_tensor("offb", [NB * 8], dt.int32, kind="ExternalInput")
    dbg = "ExternalOutput" if os.environ.get("ADA_DEBUG") == "1" else "Internal"
    u_dram = nc.dram_tensor("u_dbg", [NIMG * 3 * UH * UW + UH * UW], BF, kind=dbg)
    w2_dram = nc.dram_tensor("w2_dbg", [NIMG * 3 * WTY * WTX], BF, kind=dbg)
    out_t = nc.dram_tensor("out", [NIMG, 3, H, W], FL, kind="ExternalOutput")

    def dap(th, offset, dims):
        return bass.AP(th, int(offset), [list(d) for d in dims])

    with TileContext(nc) as tc:
        with tc.tile_pool(name="const", bufs=1) as cpool, \
             tc.tile_pool(name="psum", bufs=4, space="PSUM") as ppool:

            # ---- constants staged once ----
            scal_sb = cpool.tile([128, NB, 6], FL, tag="scal")
            nc.sync.dma_start(out=scal_sb[:, :, :], in_=dap(
                scal_t, 0, [(1, 128), (6 * 128, NB), (128, 6)]))
            iota_xf = cpool.tile([128, NIDX], BF, tag="iotaxf")
            iota_yf = cpool.tile([128, NIDX], BF, tag="iotayf")

            # single window staging buffer (guards zeroed once)
            wb0 = cpool.tile([128, WRF * WCF], BF, tag="wb0")
            nc.vector.memset(wb0[:, :], 0.0)
            wb_list = [wb0, wb0]
            # row-pair interleaved windows (double-buffered): wb2[2j]=wb[j],
            # wb2[2j+1]=wb[j+WCF] so one gather index fetches all 4 corners
            wb2a = cpool.tile([128, 2 * WRF * WCF], BF, tag="wb2a")
            nc.vector.memset(wb2a[:, :], 0.0)
            wb2b = cpool.tile([128, 2 * WRF * WCF], BF, tag="wb2b")
            nc.vector.memset(wb2b[:, :], 0.0)
            wb2_list = [wb2a, wb2b]

            # =================== phase 1: FIR up (per image) ===================
            fir_pool_ctx = tc.tile_pool(name="fir", bufs=1)
            fpool = fir_pool_ctx.__enter__()
            buy_sb = fpool.tile([128, 3, UH], FL, tag="buy")
            nc.sync.dma_start(out=buy_sb[:, :, :], in_=dap(
                buy_t, 0, [(UH, 128), (128 * UH, 3), (1, UH)]))
            iota_xi = fpool.tile([128, NIDX], dt.int32, tag="iotai")
            nc.gpsimd.iota(iota_xi[:, :], pattern=[[0, TY], [1, TX]], base=0,
                           channel_multiplier=0)
            nc.scalar.copy(out=iota_xf[:, :], in_=iota_xi[:, :])
            nc.gpsimd.iota(iota_xi[:, :], pattern=[[1, TY], [0, TX]], base=0,
                           channel_multiplier=0)
            nc.scalar.copy(out=iota_yf[:, :], in_=iota_xi[:, :])
            for img in range(NIMG):
                xpe = fpool.tile([128, 3, 3, P + 12], FL, tag="xpe")
                nc.vector.memset(xpe[:, :, :, :], 0.0)
                # load 332 rows into (blk, part): blk 0-1 full, blk 2 rows 0-75
                for blk in range(3):
                    pr = 128 if blk < 2 else P - 256
                    nc.sync.dma_start(
                        out=xpe[0:pr, blk, :, 6:6 + P],
                        in_=dap(xpad_t, img * 3 * P * P + blk * 128 * P,
                                [(P, pr), (P * P, 3), (1, P)]))
                # up-x on DVE: T1[.., par::2] = sum taps
                t1 = fpool.tile([128, 3, 3, UH], FL, tag="t1", bufs=2)
                for par in (0, 1):
                    for k, (dm, cf) in enumerate(up_taps[par]):
                        src = xpe[:, :, :, 6 + dm:6 + dm + P]
                        dst = t1[:, :, :, par::2]
                        if k == 0:
                            nc.vector.tensor_scalar(dst, src, float(cf), None, ALU.mult)
                        else:
                            nc.vector.scalar_tensor_tensor(
                                dst, src, float(cf), dst, ALU.mult, ALU.add)
                # up-y on PE: per M-tile, accumulate over K partition-blocks
                for mt in range(6):
                    ms, me = mt * 128, min(mt * 128 + 128, UH)
                    mm = me - ms
                    # K-window rows from Buy sparsity: out col j taps rows (j-par)/2+dm
                    r_lo = max(ms // 2 + min(d for d, _ in up_taps[0] + up_taps[1]), 0)
                    r_hi = min((me - 1) // 2 + max(d for d, _ in up_taps[0] + up_taps[1]), P - 1)
                    blks = list(range(r_lo // 128, r_hi // 128 + 1))
                    for ch in range(3):
                        for cs in (0, 512):
                            ce = min(cs + 512, UH)
                            nn = ce - cs
                            ps = ppool.tile([128, 512], FL, tag="ps_u")
                            for bi, b in enumerate(blks):
                                nc.tensor.matmul(
                                    ps[0:mm, 0:nn],
                                    buy_sb[:, b, ms:me],
                                    t1[:, b, ch, cs:ce],
                                    start=(bi == 0), stop=(bi == len(blks) - 1))
                            # evac + store (cast to bf16)
                            usb = fpool.tile([128, 512], BF, tag="usb", bufs=2)
                            nc.scalar.copy(out=usb[0:mm, 0:nn], in_=ps[0:mm, 0:nn])
                            nc.sync.dma_start(
                                out=dap(u_dram,
                                        (img * 3 + ch) * UH * UW + ms * UW + cs,
                                        [(UW, mm), (1, nn)]),
                                in_=usb[0:mm, 0:nn])

            fir_pool_ctx.__exit__(None, None, None)
            # =================== phase 2: warp (68 batches) ===================
            warp_pool_ctx = tc.tile_pool(name="warp", bufs=1)
            wpool = warp_pool_ctx.__enter__()
            prev_wdmas = []
            state = {}

            def head(b, tc=tc):
                # stage idx/offsets, fetch windows, build interleaved wb2.
                # Emitted one batch ahead so it overlaps batch b-1's gathers;
                # high_priority pulls it earlier in the scheduler's ordering.
                ctx = tc.high_priority(offset=80)
                ctx.__enter__()
                wb = wb_list[b % 2]
                idx_sb = wpool.tile([128, SW], dt.uint16, tag="idx", bufs=2)
                nc.scalar.dma_start(out=idx_sb[:, :], in_=dap(
                    idx_t, b * 128 * SW, [(SW, 128), (1, SW)]))
                offb_b = wpool.tile([128, 8], dt.int32, tag="offb_b", bufs=2)
                ob_dma = nc.scalar.dma_start(out=offb_b[0:1, :], in_=dap(
                    offb_t, b * 8, [(8, 1), (1, 8)]))
                if prev_wdmas:
                    bass._add_dep_helper(
                        ob_dma.ins, prev_wdmas[-1].ins, sync=True,
                        reason="offb slot reuse waits past prior register loads")
                u_ap = u_dram.ap()
                lds, vals = nc.values_load_multi_w_load_instructions(
                    offb_b[0:1, 0:8], engines=[mybir.EngineType.Activation],
                    min_val=0, max_val=(NIMG - 1) * 3 * UH * UW + UH * UW,
                    skip_runtime_bounds_check=True)
                if prev_wdmas:
                    for ld in lds:
                        bass._add_dep_helper(
                            ld.ins, prev_wdmas[-1].ins, sync=False,
                            reason="bound window-offset register liveness")
                wdmas = []
                for g in range(8):
                    src = u_ap[bass.ds(vals[g], 3 * UH * UW)].rearrange(
                        "(c r x) -> c r x", c=3, x=UW)[:, 0:WRM, 0:WCM]
                    wbv = wb[16 * g:16 * g + 3, :].rearrange(
                        "p (r c) -> p r c", c=WCF)[:, 2:2 + WRM, 2:2 + WCM]
                    wdmas.append(nc.scalar.dma_start(out=wbv, in_=src))
                # interleave: even elems <- wb[j] (ACT), odd <- wb[j+WCF] (DVE)
                wb2 = wb2_list[b % 2]
                wb2v = wb2[:, :].rearrange("p (a b) -> p a b", b=2)
                NW = WRF * WCF
                bld_e = nc.scalar.copy(
                    out=wb2v[:, :, 0:1].rearrange("p a b -> p (a b)"), in_=wb[:, :])
                bld_o = nc.vector.tensor_scalar(
                    wb2v[:, 0:NW - WCF, 1:2].rearrange("p a b -> p (a b)"),
                    wb[:, WCF:], 1.0, None, ALU.mult)
                state[b] = (idx_sb, wb2)
                ctx.__exit__(None, None, None)
                return wdmas, bld_e, bld_o

            prev_wdmas, _, _ = head(0)
            for b in range(NB):
                img, ty = b // GY, b % GY
                # force next batch's wb2 build to schedule before this batch's
                # ACT/DVE compute so it hides under this batch's gathers
                if b + 1 < NB:
                    prev_wdmas, bld_e, bld_o = head(b + 1)
                    force_after = (bld_e, bld_o)
                else:
                    force_after = (None, None)
                idx_sb, wb2 = state.pop(b)
                # weights: zs = ax*iotaX + (bx*iotaY + Cx); w = frac(zs)
                tBb = wpool.tile([128, 2, NIDX], FL, tag="tB")
                tA = wpool.tile([128, NIDX], FL, tag="tA")
                for k, (o_a, o_b, o_c) in enumerate(((0, 1, 2), (3, 4, 5))):
                    # tA = b*iotaY + C        (ACT)
                    w_act = nc.scalar.activation(
                        tA[:, :], iota_yf[:, :], ACTF.Identity,
                        bias=scal_sb[:, b, o_c:o_c + 1],
                        scale=scal_sb[:, b, o_b:o_b + 1])
                    if False:
                        pass
                    # zs = (iotaX * a) + tA   (DVE stt)
                    nc.vector.scalar_tensor_tensor(
                        tBb[:, k, :], iota_xf[:, :], scal_sb[:, b, o_a:o_a + 1],
                        tA[:, :], ALU.mult, ALU.add)
                    # zf in tA: int-cast then float-cast in place
                    tAi = tA[:, :].bitcast(dt.int32)
                    nc.scalar.copy(out=tAi, in_=tBb[:, k, :])
                    nc.scalar.copy(out=tA[:, :], in_=tAi)
                    # fr = zs - zf (in place)
                    nc.vector.tensor_tensor(tBb[:, k, :], tBb[:, k, :], tA[:, :],
                                            ALU.subtract)
                # merged fixup for both weights: w = (fr<0) + fr  -> bf16
                wv = wpool.tile([128, 2, NIDX], BF, tag="wv")
                nc.vector.scalar_tensor_tensor(
                    wv[:, :, :], tBb[:, :, :], 0.0, tBb[:, :, :], ALU.is_lt, ALU.add)
                wx_t = wv[:, 0, :]
                wy_t = wv[:, 1, :]
                # quad gathers: one idx -> (v00,v10,v01,v11); dst cap 1024 elems
                gq = wpool.tile([128, NIDX, 4], BF, tag="gq", bufs=2)
                wb2d = wb2[:, :].rearrange("p (a b) -> p a b", b=4)
                for c0 in range(0, NIDX, 256):
                    c1 = min(c0 + 256, NIDX)
                    nc.gpsimd.indirect_copy(
                        gq[:, c0:c1, :], wb2d, idx_sb[:, c0 // 16:c1 // 16],
                        True)

                def ev(t, k):
                    return t[:, :, k:k + 1].rearrange("p a b -> p (a b)")
                # y-lerp both columns, then x-lerp
                tmp0 = tBb[:, 0, :]
                tmp1 = tBb[:, 1, :]
                l0 = nc.vector.tensor_tensor(tmp0, ev(gq, 1), ev(gq, 0),
                                             ALU.subtract)
                if force_after[1] is not None:
                    bass._add_dep_helper(
                        l0.ins, force_after[1].ins, sync=False,
                        reason="schedule next-batch wb2 build first on DVE")
                nc.vector.tensor_tensor(tmp0, tmp0, wy_t, ALU.mult)
                nc.vector.tensor_tensor(tmp0, tmp0, ev(gq, 0), ALU.add)
                nc.vector.tensor_tensor(tmp1, ev(gq, 3), ev(gq, 2), ALU.subtract)
                nc.vector.tensor_tensor(tmp1, tmp1, wy_t, ALU.mult)
                nc.vector.tensor_tensor(tmp1, tmp1, ev(gq, 2), ALU.add)
                nc.vector.tensor_tensor(tmp1, tmp1, tmp0, ALU.subtract)
                nc.vector.tensor_tensor(tmp1, tmp1, wx_t, ALU.mult)
                outt = wpool.tile([128, NIDX], BF, tag="outt", bufs=2)
                nc.vector.tensor_tensor(outt[:, :], tmp1, tmp0, ALU.add)
                # store stripe: one DMA per channel covering all 8 groups
                ov = outt[:, :].rearrange("(g s) (y x) -> g s y x", s=16, x=TX)
                for ch in range(3):
                    nc.sync.dma_start(
                        out=dap(w2_dram,
                                (img * 3 + ch) * WTY * WTX + ty * TY * WTX,
                                [(66, 8), (WTX, TY), (1, TX)]),
                        in_=ov[:, ch, :, :])

            warp_pool_ctx.__exit__(None, None, None)
            # =================== phase 3: FIR down (per image) =================
            dn_pool_ctx = tc.tile_pool(name="down", bufs=2)
            fpool = dn_pool_ctx.__enter__()
            bdy_sb = fpool.tile([128, 5, H], FL, tag="bdy")
            nc.sync.dma_start(out=bdy_sb[:, :, :], in_=dap(
                bdy_t, 0, [(H, 128), (128 * H, 5), (1, H)]))
            for img in range(NIMG):
                w2e = fpool.tile([128, 5, 3, WT], BF, tag="w2e")
                nc.vector.memset(w2e[:, :, :, :], 0.0)
                for blk in range(5):
                    pr = 128 if blk < 4 else WT - 512
                    nc.sync.dma_start(
                        out=w2e[0:pr, blk, :, :],
                        in_=dap(w2_dram, img * 3 * WTY * WTX + blk * 128 * WTX,
                                [(WTX, pr), (WTY * WTX, 3), (1, WT)]))
                # down-x on DVE (stride-2 taps)
                d1 = fpool.tile([128, 5, 3, H], FL, tag="d1")
                for k, (dm, cf) in enumerate(dn_taps[0]):
                    src = w2e[:, :, :, dm:dm + 2 * H:2]
                    if k == 0:
                        nc.vector.tensor_scalar(d1[:, :, :, :], src, float(cf), None, ALU.mult)
                    else:
                        nc.vector.scalar_tensor_tensor(
                            d1[:, :, :, :], src, float(cf), d1[:, :, :, :], ALU.mult, ALU.add)
                # down-y on PE
                dlo = min(d for d, _ in dn_taps[0])
                dhi = max(d for d, _ in dn_taps[0])
                for mt in range(2):
                    ms, me = mt * 128, mt * 128 + 128
                    r_lo = max(2 * ms + dlo, 0)
                    r_hi = min(2 * (me - 1) + dhi, WT - 1)
                    blks = list(range(r_lo // 128, r_hi // 128 + 1))
                    for ch in range(3):
                        ps = ppool.tile([128, 512], FL, tag="ps_o")
                        for bi, bb in enumerate(blks):
                            nc.tensor.matmul(
                                ps[0:128, 0:H],
                                bdy_sb[:, bb, ms:me],
                                d1[:, bb, ch, :],
                                start=(bi == 0), stop=(bi == len(blks) - 1))
                        ob = fpool.tile([128, H], FL, tag="ob")
                        nc.scalar.copy(out=ob[:, :], in_=ps[:, 0:H])
                        nc.sync.dma_start(
                            out=dap(out_t, (img * 3 + ch) * H * W + ms * W,
                                    [(W, 128), (1, H)]),
                            in_=ob[:, :])
            dn_pool_ctx.__exit__(None, None, None)

    nc.compile()
    return nc


# ---------------- entry point ------------------------------------------------

def kernel(**inputs):
    from concourse import bass_utils

    images = np.asarray(inputs["images"], np.float32)
    theta = np.asarray(inputs["theta"], np.float32)
    log_s = np.asarray(inputs["log_s"], np.float32)
    tx = np.asarray(inputs["tx"], np.float32)
    ty = np.asarray(inputs["ty"], np.float32)
    hz = np.asarray(inputs["hz_geom"], np.float32)
    N = images.shape[0]
    ncores = 8
    per = N // ncores

    A = affine_params(theta, log_s, tx, ty)
    xpad = reflect_pad(images).astype(F32)
    Bux = fir_up_matrix(hz)
    Bdx = fir_down_matrix(hz)
    up_taps = tap_structure(Bux, 2)
    dn_taps = tap_structure(Bdx, 1)
    # device down-x reads w2e[:, :, :, dm : dm+2H : 2] -> offsets must be >= 0
    assert min(d for d, _ in dn_taps[0]) >= 0

    all_tiles = [plan_image(A[i]) for i in range(N)]
    WRM, WCM = window_extents(all_tiles)
    WRF, WCF = finalize_tiles(all_tiles, WRM, WCM)
    assert WRF * WCF <= 32000, (WRF, WCF)  # quad idx = 2*(r*WCF+c) must fit u16

    # pack per-core inputs
    buy_pack = np.zeros((3, 128, UH), F32)
    buy_pack.reshape(384, UH)[:P] = Bux
    bdy_pack = np.zeros((5, 128, H), F32)
    bdy_pack.reshape(640, H)[:WT] = Bdx

    in_maps = []
    for core in range(ncores):
        idx_arr = np.zeros((NB, 128, SW), np.uint16)
        scal_arr = np.zeros((NB, 6, 128), F32)
        offb_arr = np.zeros((NB * 8,), np.int32)
        for b in range(NB):
            img, tyy = b // GY, b % GY
            gi = core * per + img
            tiles = all_tiles[gi]
            for g in range(8):
                t = tiles[tyy * GX + g]
                idx_arr[b, 16 * g:16 * g + 16, 0:SW] = wrap16(t["idxA"])
                for k in range(6):
                    scal_arr[b, k, 16 * g:16 * g + 16] = t["consts"][k]
                offb_arr[b * 8 + g] = img * 3 * UH * UW + t["r0"] * UW + t["c0"]
        in_maps.append({
            "xpad": np.ascontiguousarray(xpad[core * per:(core + 1) * per]),
            "buy": buy_pack, "bdy": bdy_pack,
            "idx": idx_arr, "scal": scal_arr, "offb": offb_arr,
        })

    nc = build_graph(WRF, WCF, WRM, WCM, up_taps, dn_taps)
    res = bass_utils.run_bass_kernel_spmd(nc, in_maps, core_ids=list(range(ncores)))
    out = np.concatenate([res.results[i]["out"] for i in range(ncores)], 0)
    kernel.last_results = res
    return out



# revision 53
# speedup vs baseline: 1.0593x; 1.0026x over previous
"""AdaAugment Trainium2 kernel: reflect-pad + FIR up2 + affine bilinear warp + FIR down2.

Self-contained. Strategy (per NeuronCore, data-parallel over batch: 4 images/core):
 - host: reflect pad, banded FIR matrices, per-tile warp plans (indices/weights consts)
 - device: x-FIRs on DVE (strided taps), y-FIRs on PE (banded matmuls),
   warp via gpsimd indirect_copy gathers over DMA'd U windows, bilinear lerp on DVE,
   weights from iota + per-partition affine (bit-exact host mirror).
One SPMD graph for all 8 cores; all per-image geometry is input data.
"""
import sys, os
for p in ("/opt/trn_rl_repo", "/root/.axon_site/_ro/trn_rl_repo"):
    if os.path.isdir(p) and p not in sys.path:
        sys.path.insert(0, p)
import numpy as np

F32 = np.float32
H = W = 256
FW = 12
HZ_PAD = 3
MARGIN = 38
P = H + 2 * MARGIN            # 332
UH = UW = 664
WT = (H + 2 * HZ_PAD) * 2     # 524
TY, TX = 32, 66
GY, GX = 17, 8
WTY, WTX = GY * TY, GX * TX   # 544, 528
NIDX = TY * TX                # 2112
SW = NIDX // 16               # 132 wrapped idx cols
NB = 4 * GY                   # 68 batches per core
ZOFF = F32(1024.0)
NIMG = 4                      # images per core


# ---------------- host planning (mirrors device fp32 ops bit-exactly) --------

def affine_params(theta, log_s, tx, ty):
    N = theta.shape[0]
    s = np.exp(log_s).astype(F32)
    c, sn = np.cos(theta).astype(F32), np.sin(theta).astype(F32)
    A = np.zeros((N, 2, 3), F32)
    for i in range(N):
        rot = np.array([[c[i], sn[i], 0], [-sn[i], c[i], 0], [0, 0, 1]], F32)
        scl = np.array([[1 / s[i], 0, 0], [0, 1 / s[i], 0], [0, 0, 1]], F32)
        trn = np.array([[1, 0, -tx[i] * W], [0, 1, -ty[i] * H], [0, 0, 1]], F32)
        g = (scl @ rot @ trn).astype(F32)
        g = (np.array([[2, 0, 0], [0, 2, 0], [0, 0, 1]], F32) @ g
             @ np.array([[.5, 0, 0], [0, .5, 0], [0, 0, 1]], F32)).astype(F32)
        g = (np.array([[1, 0, -.5], [0, 1, -.5], [0, 0, 1]], F32) @ g
             @ np.array([[1, 0, .5], [0, 1, .5], [0, 0, 1]], F32)).astype(F32)
        g = (np.array([[2.0 / UW, 0, 0], [0, 2.0 / UH, 0], [0, 0, 1]], F32) @ g
             @ np.array([[WT / 2.0, 0, 0], [0, WT / 2.0, 0], [0, 0, 1]], F32)).astype(F32)
        A[i] = g[:2, :]
    return A


def pixel_affine(A):
    Ad = A.astype(np.float64)
    ax = Ad[0, 0] * UW / WT
    bx = Ad[0, 1] * UW / WT
    cx = (Ad[0, 0] * (1.0 / WT - 1.0) + Ad[0, 1] * (1.0 / WT - 1.0) + Ad[0, 2] + 1.0) * UW / 2.0 - 0.5
    ay = Ad[1, 0] * UW / WT
    by = Ad[1, 1] * UW / WT
    cy = (Ad[1, 0] * (1.0 / WT - 1.0) + Ad[1, 1] * (1.0 / WT - 1.0) + Ad[1, 2] + 1.0) * UH / 2.0 - 0.5
    return F32(ax), F32(bx), F32(cx), F32(ay), F32(by), F32(cy)


def fir_up_matrix(f):
    f2 = np.asarray(f, np.float64) * 2.0
    p0 = (FW + 1) // 2
    B = np.zeros((P, 2 * P), np.float64)
    for m in range(P):
        jlo, jhi = p0 + 2 * m - (FW - 1), p0 + 2 * m
        for j in range(max(jlo, 0), min(jhi + 1, 2 * P)):
            B[m, j] = f2[FW - 1 - (p0 + 2 * m - j)]
    return B.astype(F32)


def fir_down_matrix(f):
    fd = np.asarray(f, np.float64)
    B = np.zeros((WT, H), np.float64)
    for j in range(H):
        for t in range(FW):
            m = 2 * j + 1 + t
            if 0 <= m < WT:
                B[m, j] = fd[t]
    return B.astype(F32)


def reflect_pad(img):
    return np.pad(img, ((0, 0), (0, 0), (MARGIN, MARGIN), (MARGIN, MARGIN)), mode="reflect")


def plan_image(A):
    ax, bx, cx, ay, by, cy = pixel_affine(A)
    tiles = []
    ly = np.arange(TY, dtype=F32)[:, None]
    lx = np.arange(TX, dtype=F32)[None, :]
    for ty in range(GY):
        for tg in range(GX):
            yo0, xo0 = ty * TY, tg * TX
            Cx = F32(F32(F32(ax * xo0) + F32(bx * yo0)) + F32(cx + float(ZOFF)))
            Cy = F32(F32(F32(ay * xo0) + F32(by * yo0)) + F32(cy + float(ZOFF)))
            # device order: tA = f32(f32(ly*b) + C); z = f32(f32(lx*a) + tA)
            zx = np.float32(np.float32(lx * ax) + np.float32(np.float32(ly * bx) + Cx))
            zy = np.float32(np.float32(lx * ay) + np.float32(np.float32(ly * by) + Cy))
            wx = np.fmod(zx, F32(1.0))
            wy = np.fmod(zy, F32(1.0))
            ix0 = np.floor(zx).astype(np.int64) - int(ZOFF)
            iy0 = np.floor(zy).astype(np.int64) - int(ZOFF)
            tiles.append(dict(ty=ty, tg=tg, ix0=ix0, iy0=iy0, wx=wx, wy=wy,
                              consts=(ax, bx, Cx, ay, by, Cy)))
    return tiles


def window_extents(all_tiles):
    WRM = WCM = 8
    for tiles in all_tiles:
        for t in tiles:
            vx = (t["ix0"] >= -1) & (t["ix0"] <= UW - 1)
            vy = (t["iy0"] >= -1) & (t["iy0"] <= UH - 1)
            use = vx & vy
            if use.any():
                c0i = max(int(t["ix0"][use].min()), 0)
                c1i = min(int(t["ix0"][use].max()) + 1, UW - 1)
                r0i = max(int(t["iy0"][use].min()), 0)
                r1i = min(int(t["iy0"][use].max()) + 1, UH - 1)
                WRM = max(WRM, r1i - r0i + 1)
                WCM = max(WCM, c1i - c0i + 1)
                t["r0"], t["c0"] = r0i, c0i
            else:
                t["r0"], t["c0"] = 0, 0
    return WRM, WCM


def finalize_tiles(all_tiles, WRM, WCM):
    WRF, WCF = WRM + 4, WCM + 4
    for tiles in all_tiles:
        for t in tiles:
            r0 = min(t["r0"], UH - WRM)
            c0 = min(t["c0"], UW - WCM)
            t["r0"], t["c0"] = r0, c0
            ix0, iy0 = t["ix0"], t["iy0"]
            lc = ix0 - c0 + 2
            lr = iy0 - r0 + 2
            lc = np.where(ix0 < c0 - 1, 0, lc)
            lc = np.where(ix0 == c0 - 1, 1, lc)
            lc = np.where(ix0 > c0 + WCM - 1, WCF - 2, lc)
            lr = np.where(iy0 < r0 - 1, 0, lr)
            lr = np.where(iy0 == r0 - 1, 1, lr)
            lr = np.where(iy0 > r0 + WRM - 1, WRF - 2, lr)
            # quad index into the row-pair interleaved window (2 elems per slot)
            t["idxA"] = (2 * (lr * WCF + lc)).astype(np.uint16).ravel()
    return WRF, WCF


def wrap16(idx_flat):
    """Wrap the full index stream into [16, n/16] (single indirect_copy call)."""
    return idx_flat.reshape(idx_flat.shape[0] // 16, 16).T


def tap_structure(B, up):
    """Extract (offsets, coeffs) per output parity from a banded FIR matrix.
    up=2: out col j=2q+par taps rows q+dm; up=1(down): out col j taps rows 2j+dm."""
    taps = []
    if up == 2:
        for par in (0, 1):
            q0 = B.shape[0] // 2
            col = B[:, 2 * q0 + par]
            rows = np.nonzero(col)[0]
            taps.append([(int(r - q0), float(col[r])) for r in rows])
    else:
        j0 = B.shape[1] // 2
        col = B[:, j0]
        rows = np.nonzero(col)[0]
        taps.append([(int(r - 2 * j0), float(col[r])) for r in rows])
    return taps


# ---------------- device graph ----------------------------------------------

def build_graph(WRF, WCF, WRM, WCM, up_taps, dn_taps):
    import concourse.bass as bass
    import concourse.bacc as bacc
    import concourse.mybir as mybir
    from concourse.tile import TileContext

    dt = mybir.dt
    ALU = mybir.AluOpType
    ACTF = mybir.ActivationFunctionType
    FL = dt.float32
    BF = dt.bfloat16

    nc = bacc.Bacc("TRN2", target_bir_lowering=False, debug=False, num_devices=8)
    nc.disable_value_cache = True
    xpad_t = nc.dram_tensor("xpad", [NIMG, 3, P, P], FL, kind="ExternalInput")
    buy_t = nc.dram_tensor("buy", [3, 128, UH], FL, kind="ExternalInput")
    bdy_t = nc.dram_tensor("bdy", [5, 128, H], FL, kind="ExternalInput")
    idx_t = nc.dram_tensor("idx", [NB, 128, SW], dt.uint16, kind="ExternalInput")
    scal_t = nc.dram_tensor("scal", [NB, 6, 128], FL, kind="ExternalInput")
    offb_t = nc.dram_tensor("offb", [NB * 8], dt.int32, kind="ExternalInput")
    dbg = "ExternalOutput" if os.environ.get("ADA_DEBUG") == "1" else "Internal"
    u_dram = nc.dram_tensor("u_dbg", [NIMG * 3 * UH * UW + UH * UW], BF, kind=dbg)
    w2_dram = nc.dram_tensor("w2_dbg", [NIMG * 3 * WTY * WTX], BF, kind=dbg)
    out_t = nc.dram_tensor("out", [NIMG, 3, H, W], FL, kind="ExternalOutput")

    def dap(th, offset, dims):
        return bass.AP(th, int(offset), [list(d) for d in dims])

    with TileContext(nc) as tc:
        with tc.tile_pool(name="const", bufs=1) as cpool, \
             tc.tile_pool(name="psum", bufs=4, space="PSUM") as ppool:

            # ---- constants staged once ----
            scal_sb = cpool.tile([128, NB, 6], FL, tag="scal")
            nc.sync.dma_start(out=scal_sb[:, :, :], in_=dap(
                scal_t, 0, [(1, 128), (6 * 128, NB), (128, 6)]))
            iota_xf = cpool.tile([128, NIDX], BF, tag="iotaxf")
            iota_yf = cpool.tile([128, NIDX], BF, tag="iotayf")

            # single window staging buffer (guards zeroed once)
            wb0 = cpool.tile([128, WRF * WCF], BF, tag="wb0")
            nc.vector.memset(wb0[:, :], 0.0)
            wb_list = [wb0, wb0]
            # row-pair interleaved windows (double-buffered): wb2[2j]=wb[j],
            # wb2[2j+1]=wb[j+WCF] so one gather index fetches all 4 corners
            wb2a = cpool.tile([128, 2 * WRF * WCF], BF, tag="wb2a")
            nc.vector.memset(wb2a[:, :], 0.0)
            wb2b = cpool.tile([128, 2 * WRF * WCF], BF, tag="wb2b")
            nc.vector.memset(wb2b[:, :], 0.0)
            wb2_list = [wb2a, wb2b]

            # =================== phase 1: FIR up (per image) ===================
            fir_pool_ctx = tc.tile_pool(name="fir", bufs=1)
            fpool = fir_pool_ctx.__enter__()
            buy_sb = fpool.tile([128, 3, UH], FL, tag="buy")
            nc.sync.dma_start(out=buy_sb[:, :, :], in_=dap(
                buy_t, 0, [(UH, 128), (128 * UH, 3), (1, UH)]))
            iota_xi = fpool.tile([128, NIDX], dt.int32, tag="iotai")
            nc.gpsimd.iota(iota_xi[:, :], pattern=[[0, TY], [1, TX]], base=0,
                           channel_multiplier=0)
            nc.scalar.copy(out=iota_xf[:, :], in_=iota_xi[:, :])
            nc.gpsimd.iota(iota_xi[:, :], pattern=[[1, TY], [0, TX]], base=0,
                           channel_multiplier=0)
            nc.scalar.copy(out=iota_yf[:, :], in_=iota_xi[:, :])
            for img in range(NIMG):
                xpe = fpool.tile([128, 3, 3, P + 12], FL, tag="xpe")
                nc.vector.memset(xpe[:, :, :, :], 0.0)
                # load 332 rows into (blk, part): blk 0-1 full, blk 2 rows 0-75
                for blk in range(3):
                    pr = 128 if blk < 2 else P - 256
                    nc.sync.dma_start(
                        out=xpe[0:pr, blk, :, 6:6 + P],
                        in_=dap(xpad_t, img * 3 * P * P + blk * 128 * P,
                                [(P, pr), (P * P, 3), (1, P)]))
                # up-x on DVE: T1[.., par::2] = sum taps
                t1 = fpool.tile([128, 3, 3, UH], FL, tag="t1", bufs=2)
                for par in (0, 1):
                    for k, (dm, cf) in enumerate(up_taps[par]):
                        src = xpe[:, :, :, 6 + dm:6 + dm + P]
                        dst = t1[:, :, :, par::2]
                        if k == 0:
                            nc.vector.tensor_scalar(dst, src, float(cf), None, ALU.mult)
                        else:
                            nc.vector.scalar_tensor_tensor(
                                dst, src, float(cf), dst, ALU.mult, ALU.add)
                # up-y on PE: per M-tile, accumulate over K partition-blocks
                for mt in range(6):
                    ms, me = mt * 128, min(mt * 128 + 128, UH)
                    mm = me - ms
                    # K-window rows from Buy sparsity: out col j taps rows (j-par)/2+dm
                    r_lo = max(ms // 2 + min(d for d, _ in up_taps[0] + up_taps[1]), 0)
                    r_hi = min((me - 1) // 2 + max(d for d, _ in up_taps[0] + up_taps[1]), P - 1)
                    blks = list(range(r_lo // 128, r_hi // 128 + 1))
                    for ch in range(3):
                        for cs in (0, 512):
                            ce = min(cs + 512, UH)
                            nn = ce - cs
                            ps = ppool.tile([128, 512], FL, tag="ps_u")
                            for bi, b in enumerate(blks):
                                nc.tensor.matmul(
                                    ps[0:mm, 0:nn],
                                    buy_sb[:, b, ms:me],
                                    t1[:, b, ch, cs:ce],
                                    start=(bi == 0), stop=(bi == len(blks) - 1))
                            # evac + store (cast to bf16)
                            usb = fpool.tile([128, 512], BF, tag="usb", bufs=2)
                            nc.scalar.copy(out=usb[0:mm, 0:nn], in_=ps[0:mm, 0:nn])
                            nc.sync.dma_start(
                                out=dap(u_dram,
                                        (img * 3 + ch) * UH * UW + ms * UW + cs,
                                        [(UW, mm), (1, nn)]),
                                in_=usb[0:mm, 0:nn])

            fir_pool_ctx.__exit__(None, None, None)
            # =================== phase 2: warp (68 batches) ===================
            warp_pool_ctx = tc.tile_pool(name="warp", bufs=1)
            wpool = warp_pool_ctx.__enter__()
            prev_wdmas = []
            state = {}

            def head(b, tc=tc):
                # stage idx/offsets, fetch windows, build interleaved wb2.
                # Emitted one batch ahead so it overlaps batch b-1's gathers;
                # high_priority pulls it earlier in the scheduler's ordering.
                ctx = tc.high_priority(offset=80)
                ctx.__enter__()
                wb = wb_list[b % 2]
                idx_sb = wpool.tile([128, SW], dt.uint16, tag="idx", bufs=2)
                nc.scalar.dma_start(out=idx_sb[:, :], in_=dap(
                    idx_t, b * 128 * SW, [(SW, 128), (1, SW)]))
                offb_b = wpool.tile([128, 8], dt.int32, tag="offb_b", bufs=2)
                ob_dma = nc.scalar.dma_start(out=offb_b[0:1, :], in_=dap(
                    offb_t, b * 8, [(8, 1), (1, 8)]))
                if prev_wdmas:
                    bass._add_dep_helper(
                        ob_dma.ins, prev_wdmas[-1].ins, sync=True,
                        reason="offb slot reuse waits past prior register loads")
                u_ap = u_dram.ap()
                lds, vals = nc.values_load_multi_w_load_instructions(
                    offb_b[0:1, 0:8], engines=[mybir.EngineType.Activation],
                    min_val=0, max_val=(NIMG - 1) * 3 * UH * UW + UH * UW,
                    skip_runtime_bounds_check=True)
                if prev_wdmas:
                    for ld in lds:
                        bass._add_dep_helper(
                            ld.ins, prev_wdmas[-1].ins, sync=False,
                            reason="bound window-offset register liveness")
                wdmas = []
                for g in range(8):
                    src = u_ap[bass.ds(vals[g], 3 * UH * UW)].rearrange(
                        "(c r x) -> c r x", c=3, x=UW)[:, 0:WRM, 0:WCM]
                    wbv = wb[16 * g:16 * g + 3, :].rearrange(
                        "p (r c) -> p r c", c=WCF)[:, 2:2 + WRM, 2:2 + WCM]
                    wdmas.append(nc.scalar.dma_start(out=wbv, in_=src))
                # interleave: even elems <- wb[j] (ACT), odd <- wb[j+WCF] (DVE)
                wb2 = wb2_list[b % 2]
                wb2v = wb2[:, :].rearrange("p (a b) -> p a b", b=2)
                NW = WRF * WCF
                bld_e = nc.scalar.copy(
                    out=wb2v[:, :, 0:1].rearrange("p a b -> p (a b)"), in_=wb[:, :])
                bld_o = nc.vector.tensor_scalar(
                    wb2v[:, 0:NW - WCF, 1:2].rearrange("p a b -> p (a b)"),
                    wb[:, WCF:], 1.0, None, ALU.mult)
                state[b] = (idx_sb, wb2)
                ctx.__exit__(None, None, None)
                return wdmas, bld_e, bld_o

            prev_wdmas, _, _ = head(0)
            for b in range(NB):
                img, ty = b // GY, b % GY
                # force next batch's wb2 build to schedule before this batch's
                # ACT/DVE compute so it hides under this batch's gathers
                if b + 1 < NB:
                    prev_wdmas, bld_e, bld_o = head(b + 1)
                    force_after = (bld_e, bld_o)
                else:
                    force_after = (None, None)
                idx_sb, wb2 = state.pop(b)
                # weights: zs = ax*iotaX + (bx*iotaY + Cx); w = frac(zs)
                tBb = wpool.tile([128, 2, NIDX], FL, tag="tB")
                tA = wpool.tile([128, NIDX], FL, tag="tA")
                for k, (o_a, o_b, o_c) in enumerate(((0, 1, 2), (3, 4, 5))):
                    # tA = b*iotaY + C        (ACT)
                    w_act = nc.scalar.activation(
                        tA[:, :], iota_yf[:, :], ACTF.Identity,
                        bias=scal_sb[:, b, o_c:o_c + 1],
                        scale=scal_sb[:, b, o_b:o_b + 1])
                    if False:
                        pass
                    # zs = (iotaX * a) + tA   (DVE stt)
                    nc.vector.scalar_tensor_tensor(
                        tBb[:, k, :], iota_xf[:, :], scal_sb[:, b, o_a:o_a + 1],
                        tA[:, :], ALU.mult, ALU.add)
                    # zf in tA: int-cast then float-cast in place
                    tAi = tA[:, :].bitcast(dt.int32)
                    nc.scalar.copy(out=tAi, in_=tBb[:, k, :])
                    nc.scalar.copy(out=tA[:, :], in_=tAi)
                    # fr = zs - zf (in place)
                    nc.vector.tensor_tensor(tBb[:, k, :], tBb[:, k, :], tA[:, :],
                                            ALU.subtract)
                # merged fixup for both weights: w = (fr<0) + fr  -> bf16
                wv = wpool.tile([128, 2, NIDX], BF, tag="wv")
                nc.vector.scalar_tensor_tensor(
                    wv[:, :, :], tBb[:, :, :], 0.0, tBb[:, :, :], ALU.is_lt, ALU.add)
                wx_t = wv[:, 0, :]
                wy_t = wv[:, 1, :]
                # quad gathers: one idx -> (v00,v10,v01,v11); dst cap 1024 elems
                gq = wpool.tile([128, NIDX, 4], BF, tag="gq", bufs=2)
                wb2d = wb2[:, :].rearrange("p (a b) -> p a b", b=4)
                for c0 in list(range(0, NIDX, 256))[::-1]:
                    c1 = min(c0 + 256, NIDX)
                    nc.gpsimd.indirect_copy(
                        gq[:, c0:c1, :], wb2d, idx_sb[:, c0 // 16:c1 // 16],
                        True)

                def ev(t, k):
                    return t[:, :, k:k + 1].rearrange("p a b -> p (a b)")
                # y-lerp both columns, then x-lerp
                tmp0 = tBb[:, 0, :]
                tmp1 = tBb[:, 1, :]
                l0 = nc.vector.tensor_tensor(tmp0, ev(gq, 1), ev(gq, 0),
                                             ALU.subtract)
                if force_after[1] is not None:
                    bass._add_dep_helper(
                        l0.ins, force_after[1].ins, sync=False,
                        reason="schedule next-batch wb2 build first on DVE")
                nc.vector.tensor_tensor(tmp0, tmp0, wy_t, ALU.mult)
                nc.vector.tensor_tensor(tmp0, tmp0, ev(gq, 0), ALU.add)
                nc.vector.tensor_tensor(tmp1, ev(gq, 3), ev(gq, 2), ALU.subtract)
                nc.vector.tensor_tensor(tmp1, tmp1, wy_t, ALU.mult)
                nc.vector.tensor_tensor(tmp1, tmp1, ev(gq, 2), ALU.add)
                nc.vector.tensor_tensor(tmp1, tmp1, tmp0, ALU.subtract)
                nc.vector.tensor_tensor(tmp1, tmp1, wx_t, ALU.mult)
                outt = wpool.tile([128, NIDX], BF, tag="outt", bufs=2)
                nc.vector.tensor_tensor(outt[:, :], tmp1, tmp0, ALU.add)
                # store stripe: one DMA per channel covering all 8 groups
                ov = outt[:, :].rearrange("(g s) (y x) -> g s y x", s=16, x=TX)
                for ch in range(3):
                    nc.sync.dma_start(
                        out=dap(w2_dram,
                                (img * 3 + ch) * WTY * WTX + ty * TY * WTX,
                                [(66, 8), (WTX, TY), (1, TX)]),
                        in_=ov[:, ch, :, :])

            warp_pool_ctx.__exit__(None, None, None)
            # =================== phase 3: FIR down (per image) =================
            dn_pool_ctx = tc.tile_pool(name="down", bufs=2)
            fpool = dn_pool_ctx.__enter__()
            bdy_sb = fpool.tile([128, 5, H], FL, tag="bdy")
            nc.sync.dma_start(out=bdy_sb[:, :, :], in_=dap(
                bdy_t, 0, [(H, 128), (128 * H, 5), (1, H)]))
            for img in range(NIMG):
                w2e = fpool.tile([128, 5, 3, WT], BF, tag="w2e")
                nc.vector.memset(w2e[:, :, :, :], 0.0)
                for blk in range(5):
                    pr = 128 if blk < 4 else WT - 512
                    nc.sync.dma_start(
                        out=w2e[0:pr, blk, :, :],
                        in_=dap(w2_dram, img * 3 * WTY * WTX + blk * 128 * WTX,
                                [(WTX, pr), (WTY * WTX, 3), (1, WT)]))
                # down-x on DVE (stride-2 taps)
                d1 = fpool.tile([128, 5, 3, H], FL, tag="d1")
                for k, (dm, cf) in enumerate(dn_taps[0]):
                    src = w2e[:, :, :, dm:dm + 2 * H:2]
                    if k == 0:
                        nc.vector.tensor_scalar(d1[:, :, :, :], src, float(cf), None, ALU.mult)
                    else:
                        nc.vector.scalar_tensor_tensor(
                            d1[:, :, :, :], src, float(cf), d1[:, :, :, :], ALU.mult, ALU.add)
                # down-y on PE
                dlo = min(d for d, _ in dn_taps[0])
                dhi = max(d for d, _ in dn_taps[0])
                for mt in range(2):
                    ms, me = mt * 128, mt * 128 + 128
                    r_lo = max(2 * ms + dlo, 0)
                    r_hi = min(2 * (me - 1) + dhi, WT - 1)
                    blks = list(range(r_lo // 128, r_hi // 128 + 1))
                    for ch in range(3):
                        ps = ppool.tile([128, 512], FL, tag="ps_o")
                        for bi, bb in enumerate(blks):
                            nc.tensor.matmul(
                                ps[0:128, 0:H],
                                bdy_sb[:, bb, ms:me],
                                d1[:, bb, ch, :],
                                start=(bi == 0), stop=(bi == len(blks) - 1))
                        ob = fpool.tile([128, H], FL, tag="ob")
                        nc.scalar.copy(out=ob[:, :], in_=ps[:, 0:H])
                        nc.sync.dma_start(
                            out=dap(out_t, (img * 3 + ch) * H * W + ms * W,
                                    [(W, 128), (1, H)]),
                            in_=ob[:, :])
            dn_pool_ctx.__exit__(None, None, None)

    nc.compile()
    return nc


# ---------------- entry point ------------------------------------------------

def kernel(**inputs):
    from concourse import bass_utils

    images = np.asarray(inputs["images"], np.float32)
    theta = np.asarray(inputs["theta"], np.float32)
    log_s = np.asarray(inputs["log_s"], np.float32)
    tx = np.asarray(inputs["tx"], np.float32)
    ty = np.asarray(inputs["ty"], np.float32)
    hz = np.asarray(inputs["hz_geom"], np.float32)
    N = images.shape[0]
    ncores = 8
    per = N // ncores

    A = affine_params(theta, log_s, tx, ty)
    xpad = reflect_pad(images).astype(F32)
    Bux = fir_up_matrix(hz)
    Bdx = fir_down_matrix(hz)
    up_taps = tap_structure(Bux, 2)
    dn_taps = tap_structure(Bdx, 1)
    # device down-x reads w2e[:, :, :, dm : dm+2H : 2] -> offsets must be >= 0
    assert min(d for d, _ in dn_taps[0]) >= 0

    all_tiles = [plan_image(A[i]) for i in range(N)]
    WRM, WCM = window_extents(all_tiles)
    WRF, WCF = finalize_tiles(all_tiles, WRM, WCM)
    assert WRF * WCF <= 32000, (WRF, WCF)  # quad idx = 2*(r*WCF+c) must fit u16

    # pack per-core inputs
    buy_pack = np.zeros((3, 128, UH), F32)
    buy_pack.reshape(384, UH)[:P] = Bux
    bdy_pack = np.zeros((5, 128, H), F32)
    bdy_pack.reshape(640, H)[:WT] = Bdx

    in_maps = []
    for core in range(ncores):
        idx_arr = np.zeros((NB, 128, SW), np.uint16)
        scal_arr = np.zeros((NB, 6, 128), F32)
        offb_arr = np.zeros((NB * 8,), np.int32)
        for b in range(NB):
            img, tyy = b // GY, b % GY
            gi = core * per + img
            tiles = all_tiles[gi]
            for g in range(8):
                t = tiles[tyy * GX + g]
                idx_arr[b, 16 * g:16 * g + 16, 0:SW] = wrap16(t["idxA"])
                for k in range(6):
                    scal_arr[b, k, 16 * g:16 * g + 16] = t["consts"][k]
                offb_arr[b * 8 + g] = img * 3 * UH * UW + t["r0"] * UW + t["c0"]
        in_maps.append({
            "xpad": np.ascontiguousarray(xpad[core * per:(core + 1) * per]),
            "buy": buy_pack, "bdy": bdy_pack,
            "idx": idx_arr, "scal": scal_arr, "offb": offb_arr,
        })

    nc = build_graph(WRF, WCF, WRM, WCM, up_taps, dn_taps)
    res = bass_utils.run_bass_kernel_spmd(nc, in_maps, core_ids=list(range(ncores)))
    out = np.concatenate([res.results[i]["out"] for i in range(ncores)], 0)
    kernel.last_results = res
    return out

